# revision 1
# baseline (speedup 1.0000x reference)
"""MultiHeadAttention forward on 8 Trainium2 NeuronCores.

Sharding (Megatron-style tensor parallel x data parallel):
  core c (0..7): batch b = c // 4, head group g = c % 4 (4 of 16 heads).
  Wq/Wk/Wv column-sharded ([1024, 256] per core), Wo row-sharded
  ([256, 1024] per core). Each core computes a partial output
  [S, D] = attn(heads g) @ Wo_rows; the host sums the 4 partials per
  batch and adds bo (the "all-reduce" runs on host since full outputs
  are gathered anyway).

Device dataflow per core (all matmuls in float32r, full PE rate):
  QT/KT [dk-on-partition, S] via lhsT=W chunks, rhs=x^T chunks
  V natural [S-on-partition, 256] (+bias via ones-row matmul,
    +ones column appended for the softmax denominator)
  scores^T[k, q] per (head, q-block): lhsT=KT chunk, rhs=QT block
  E = exp(scores^T / sqrt(S)) via ACT, PSUM->SBUF
  attn^T [65, q] = accumulate lhsT=[V_h|1], rhs=E chunks
    (row 64 = softmax denominator; normalize by its reciprocal)
  O [q, 1024] = lhsT=attn^T chunks, rhs=Wo chunks; DMA PSUM->DRAM
"""

import math

import numpy as np

import concourse.bass as bass
import concourse.mybir as mybir
import concourse.tile as tile
from concourse import bacc
from concourse.bass_utils import run_bass_kernel_spmd

P = 128
B, S, D, H = 2, 2048, 1024, 16
NCORES = 8
GROUPS = NCORES // B          # 4 head-groups
HPC = H // GROUPS             # 4 heads per core
DK = D // H                   # 64
CPC = HPC * DK                # 256 cols per core
NP = CPC // P                 # 2 head pairs per core
DC = D // P                   # 8 contraction chunks over D
QB = 512                      # q block (matmul moving free dim)

F32 = mybir.dt.float32
F32R = mybir.dt.float32r

# DVE rejects partition-step-0 reads, so the normalize uses a
# matmul-based partition broadcast of the reciprocal row.
USE_PARTITION_BCAST = False


def build_program(seq=S):
    KT = seq // P             # k tiles
    NJ = seq // QB            # q blocks
    inv_sqrt_s = 1.0 / math.sqrt(S)  # reference scales by sqrt(full S) always

    nc = bacc.Bacc("TRN2", target_bir_lowering=False, debug=False,
                   num_devices=NCORES)
    xqT = nc.declare_dram_parameter("xqT", [D, seq], F32R, isOutput=False)
    xkT = nc.declare_dram_parameter("xkT", [D, seq], F32R, isOutput=False)
    xvT = nc.declare_dram_parameter("xvT", [D, seq], F32R, isOutput=False)
    wq = nc.declare_dram_parameter("wq", [D, CPC], F32R, isOutput=False)
    wk = nc.declare_dram_parameter("wk", [D, CPC], F32R, isOutput=False)
    wv = nc.declare_dram_parameter("wv", [D, CPC], F32R, isOutput=False)
    wo = nc.declare_dram_parameter("wo", [CPC, D], F32R, isOutput=False)
    bq = nc.declare_dram_parameter("bq", [CPC], F32, isOutput=False)
    bk = nc.declare_dram_parameter("bk", [CPC], F32, isOutput=False)
    bv = nc.declare_dram_parameter("bv", [1, CPC], F32R, isOutput=False)
    # float32r constants (memset can't write f32r: walrus ISA check)
    ones_row = nc.declare_dram_parameter("ones_row", [1, P], F32R,
                                         isOutput=False)
    vones = nc.declare_dram_parameter("vones", [P, KT * HPC], F32R,
                                      isOutput=False)
    out = nc.declare_dram_parameter("out", [seq, D], F32, isOutput=True)

    xqT_r = xqT.rearrange("(dc p) s -> p dc s", p=P)
    xkT_r = xkT.rearrange("(dc p) s -> p dc s", p=P)
    xvT_r = xvT.rearrange("(dc p) s -> p dc s", p=P)

    with tile.TileContext(nc) as tc:
        with tc.tile_pool(name="consts", bufs=1) as consts:
            bq_sb = consts.tile([P, NP], F32)
            bk_sb = consts.tile([P, NP], F32)
            bv_sb = consts.tile([1, CPC], F32R)
            ones_sb = consts.tile([1, P], F32R)
            # weight DMAs are emitted in consumption order (wk with stage-1
            # K, wq with Q, wv before V, wo before the attention pipeline)
            # so the x chunk DMAs are not queued behind cold weights.
            nc.sync.dma_start(bk_sb[:], bk.rearrange("(j p) -> p j", p=P))
            nc.sync.dma_start(bq_sb[:], bq.rearrange("(j p) -> p j", p=P))

            # Persistent activations. Per-pair / per-block tiles keep Tile's
            # dependency tracking fine-grained (stage overlap).
            qt_t = [[consts.tile([P, QB], F32R, name=f"qt_{j}_{qc}")
                     for qc in range(NJ)] for j in range(NP)]
            kt_p = [consts.tile([P, seq], F32R, name=f"kt_p{j}")
                    for j in range(NP)]
            v_sb = consts.tile([P, KT, HPC, DK + 1], F32R)
            at_j = [consts.tile([P, NP, QB], F32R, name=f"at_j{j}")
                    for j in range(NJ)]

            # ---- Stage 1 ----
            def emit_proj(name, x_r, w_src, b_sb, dst_fn, use_dve=None):
                with (
                    tc.tile_pool(name=f"xc_{name}", bufs=6) as xp,
                    tc.tile_pool(name=f"wp_{name}", bufs=1) as wp,
                    tc.tile_pool(name=f"ps_{name}", bufs=2 * NJ,
                                 space="PSUM") as psp,
                ):
                    # per-dc weight tiles, each DMA'd right after its x
                    # chunk: the dc=0 matmuls start after ~1.1MB instead of
                    # waiting for the whole weight matrix.
                    w_r = w_src.rearrange("(dc p) c -> p dc c", p=P)
                    w_dc = [wp.tile([P, CPC], F32R, tag=f"w{dc}",
                                    name=f"w_{name}{dc}")
                            for dc in range(DC)]
                    psq = [psp.tile([P, QB], F32, tag="qk", name=f"psq_{i}")
                           for i in range(NP * NJ)]
                    for dc in range(DC):
                        xt = xp.tile([P, seq], F32R, tag="xc")
                        nc.sync.dma_start(xt[:], x_r[:, dc])
                        nc.sync.dma_start(w_dc[dc][:], w_r[:, dc])
                        for j in range(NP):
                            for qc in range(NJ):
                                nc.tensor.matmul(
                                    psq[j * NJ + qc][:],
                                    w_dc[dc][:, j * P:(j + 1) * P],
                                    xt[:, qc * QB:(qc + 1) * QB],
                                    start=(dc == 0), stop=(dc == DC - 1),
                                )
                    for j in range(NP):
                        for qc in range(NJ):
                            # bias-add psum drains: ACT (idle in stage 1)
                            # except pairs routed to DVE to keep the first
                            # exps from queueing behind them on ACT.
                            if use_dve is not None and use_dve(j, qc):
                                nc.vector.tensor_scalar_add(
                                    dst_fn(j, qc),
                                    psq[j * NJ + qc][:],
                                    b_sb[:, j:j + 1],
                                )
                            else:
                                nc.scalar.activation(
                                    dst_fn(j, qc),
                                    psq[j * NJ + qc][:],
                                    mybir.ActivationFunctionType.Identity,
                                    bias=b_sb[:, j:j + 1],
                                )

            # V projection, streamed like K/Q (dc-outer over 16 kt psums,
            # two kt sharing each psum bank) so PE tracks the x_v DMAs.
            # Emitted as a function so it can be placed AFTER the first
            # lookahead scores blocks: the in-order PE queue then has exp
            # feedstock issued ahead of the xv-DMA-paced V matmuls, and ACT
            # works through early softmax blocks during the x_v stream.
            def emit_v_block():
                with (
                    tc.tile_pool(name="xc_v", bufs=6) as xvp,
                    tc.tile_pool(name="wp_v", bufs=1) as wvp,
                    tc.tile_pool(name="ps_v", bufs=KT // 2,
                                 space="PSUM") as psvp,
                ):
                    wv_r = wv.rearrange("(dc p) c -> p dc c", p=P)
                    wv_dc = [wvp.tile([P, CPC], F32R, tag=f"w{dc}",
                                      name=f"w_v{dc}")
                             for dc in range(DC)]
                    psv = [psvp.tile([P, 2, CPC], F32, tag="v",
                                     name=f"psv_{k2}")
                           for k2 in range(KT // 2)]
                    # two kt share a psum bank => one accumulation group per
                    # bank: start only zeroes on the very first write, stop
                    # on the last bias matmul of the pair.
                    for dc in range(DC):
                        xt = xvp.tile([P, seq], F32R, tag="xc")
                        nc.sync.dma_start(xt[:], xvT_r[:, dc])
                        nc.sync.dma_start(wv_dc[dc][:], wv_r[:, dc])
                        for kt in range(KT):
                            nc.tensor.matmul(
                                psv[kt // 2][:, kt % 2],
                                xt[:, kt * P:(kt + 1) * P],
                                wv_dc[dc][:, :],
                                start=(dc == 0 and kt % 2 == 0), stop=False,
                            )
                        if dc == 0:
                            # constants are only needed at the bias/drain
                            # step; emitting them after xv0/wv0 keeps V's
                            # first matmuls off the DMA critical path.
                            nc.sync.dma_start(bv_sb[:], bv[:])
                            nc.sync.dma_start(ones_sb[:], ones_row[:])
                            with nc.allow_non_contiguous_dma(
                                    reason="one-time 32KB ones-column init"):
                                nc.sync.dma_start(
                                    v_sb[:, :, :, DK:DK + 1],
                                    vones.rearrange(
                                        "p (kt h) -> p kt h",
                                        kt=KT, h=HPC)[:, :, :, None],
                                )
                    for k2 in range(KT // 2):
                        for half in range(2):
                            nc.tensor.matmul(  # += ones^T @ bv  (bias add)
                                psv[k2][:, half], ones_sb[:], bv_sb[:],
                                start=False, stop=(half == 1),
                            )
                        for half in range(2):
                            # one strided copy per k-tile: [128, 4, 64] dest
                            # (skipping the ones column)
                            nc.vector.tensor_copy(
                                v_sb[:, 2 * k2 + half, :, 0:DK],
                                psv[k2][:, half].rearrange(
                                    "p (h d) -> p h d", h=HPC),
                            )

            # Emission order K -> V -> Q: V's xv-DMA-paced matmuls fill the
            # PE while x_q is still streaming, and V is complete before the
            # first attnV -- no V-wall inside the attention pipeline.
            emit_proj("k", xkT_r, wk, bk_sb,
                      lambda j, qc: kt_p[j][:, qc * QB:(qc + 1) * QB])
            emit_v_block()
            emit_proj("q", xqT_r, wq, bq_sb,
                      lambda j, qc: qt_t[j][qc][:],
                      use_dve=lambda j, qc: j == 1)

            # ---- Stages 2+3: pipelined attention + output projection ----
            # Per (j, h) block: scores^T + exp; attnV trails LOOKAHEAD
            # blocks. O projection for q-block j is emitted right after its
            # last head's attnV.
            K2 = KT // 2  # two score k-tiles share one psum / exp op
            LOOKAHEAD = 2
            blocks = [(j, h) for j in range(NJ) for h in range(HPC)]

            def make_emit_scores(pool):
                def emit_scores(j, h, e2):
                    hp, hj = h % 2, h // 2
                    prow = slice(hp * DK, (hp + 1) * DK)
                    for k2 in range(K2):
                        pss = pool.tile([P, 2 * QB], F32, tag="s",
                                        name=f"pss_{j}_{h}_{k2}")
                        for half in range(2):
                            kt = 2 * k2 + half
                            nc.tensor.matmul(
                                pss[:, half * QB:(half + 1) * QB],
                                kt_p[hj][prow, kt * P:(kt + 1) * P],
                                qt_t[hj][j][prow, :],
                                start=True, stop=True,
                            )
                        nc.scalar.activation(
                            e2[:, k2], pss[:],
                            mybir.ActivationFunctionType.Exp,
                            scale=inv_sqrt_s,
                        )
                return emit_scores

            with tc.tile_pool(name="epool", bufs=3) as ep:
                pending = []
                with (
                    tc.tile_pool(name="rpool", bufs=4) as rp,
                    tc.tile_pool(name="opool", bufs=4) as op,
                    tc.tile_pool(name="wop", bufs=1) as wop,
                    tc.tile_pool(name="ps_s", bufs=2, space="PSUM") as pss_p,
                    tc.tile_pool(name="ps_a", bufs=1, space="PSUM") as psa_p,
                    tc.tile_pool(name="ps_p", bufs=1, space="PSUM") as psp_p,
                    tc.tile_pool(name="ps_o", bufs=2, space="PSUM") as pso_p,
                ):
                    wo_sb = wop.tile([P, NP, D], F32R)
                    nc.sync.dma_start(wo_sb[:],
                                      wo.rearrange("(dj p) n -> p dj n", p=P))
                    emit_scores = make_emit_scores(pss_p)

                    def emit_attnv(j, h, e2):
                        hp, hj = h % 2, h // 2
                        prow = slice(hp * DK, (hp + 1) * DK)
                        psa = psa_p.tile([P, QB], F32, tag="a",
                                         name=f"psa_{j}_{h}")
                        for kt in range(KT):
                            nc.tensor.matmul(
                                psa[:DK + 1],
                                v_sb[:, kt, h, :],
                                e2[:, kt // 2,
                                   (kt % 2) * QB:(kt % 2 + 1) * QB],
                                start=(kt == 0), stop=(kt == KT - 1),
                            )
                        rc = rp.tile([1, QB], F32R, tag="rc")
                        with nc.allow_low_precision(
                                reason="f32r reciprocal for matmul bcast"):
                            nc.vector.reciprocal(rc[:], psa[DK:DK + 1, :])
                        prc = psp_p.tile([P, QB], F32, tag="p",
                                         name=f"prc_{j}_{h}")
                        nc.tensor.matmul(
                            prc[:DK], ones_sb[:, :DK], rc[:],
                            start=True, stop=True,
                        )
                        atmp = rp.tile([DK, QB], F32, tag="atmp")
                        nc.vector.tensor_copy(atmp[:], psa[:DK])
                        nc.vector.tensor_tensor(
                            at_j[j][prow, hj, :], atmp[:], prc[:DK],
                            mybir.AluOpType.mult,
                        )

                    def emit_oproj(j):
                        for ql in range(QB // P):
                            qt0 = j * (QB // P) + ql
                            o_sb = op.tile([P, D], F32, tag="o_sb")
                            for nh in range(D // QB):
                                pso = pso_p.tile([P, QB], F32, tag="o",
                                                 name=f"pso_{qt0}_{nh}")
                                for dj in range(NP):
                                    nc.tensor.matmul(
                                        pso[:],
                                        at_j[j][:, dj, ql * P:(ql + 1) * P],
                                        wo_sb[:, dj,
                                              nh * QB:(nh + 1) * QB],
                                        start=(dj == 0),
                                        stop=(dj == NP - 1),
                                    )
                                nc.vector.tensor_copy(
                                    o_sb[:, nh * QB:(nh + 1) * QB], pso[:])
                                # flush each half as soon as its copy lands
                                nc.sync.dma_start(
                                    out[qt0 * P:(qt0 + 1) * P,
                                        nh * QB:(nh + 1) * QB],
                                    o_sb[:, nh * QB:(nh + 1) * QB],
                                )

                    for (j, h) in blocks:
                        e2 = ep.tile([P, K2, 2 * QB], F32R, tag="E",
                                     name=f"e2_{j}_{h}")
                        emit_scores(j, h, e2)
                        pending.append((j, h, e2))
                        if len(pending) > LOOKAHEAD:
                            jj, hh, ee = pending.pop(0)
                            emit_attnv(jj, hh, ee)
                            if hh == HPC - 1:
                                emit_oproj(jj)
                    for jj, hh, ee in pending:
                        emit_attnv(jj, hh, ee)
                        if hh == HPC - 1:
                            emit_oproj(jj)

    nc.compile()
    return nc


_PROGRAM_CACHE = {}


def _get_program(seq=S):
    if seq not in _PROGRAM_CACHE:
        _PROGRAM_CACHE[seq] = build_program(seq)
    return _PROGRAM_CACHE[seq]


def make_in_maps(queries, keys, values, Wq, bq, Wk, bk, Wv, bv, Wo, bo):
    """Per-core input dicts implementing the sharding."""
    f32 = np.float32
    seq = np.asarray(queries).shape[1]
    xT = {}
    for b in range(B):
        xT[b] = (
            np.ascontiguousarray(np.asarray(queries[b], dtype=f32).T),
            np.ascontiguousarray(np.asarray(keys[b], dtype=f32).T),
            np.ascontiguousarray(np.asarray(values[b], dtype=f32).T),
        )
    Wq, Wk, Wv, Wo = (np.asarray(a, dtype=f32) for a in (Wq, Wk, Wv, Wo))
    bq, bk, bv = (np.asarray(a, dtype=f32) for a in (bq, bk, bv))
    in_maps = []
    for c in range(NCORES):
        b, g = divmod(c, GROUPS)
        cs = slice(g * CPC, (g + 1) * CPC)
        qT, kT, vT = xT[b]
        in_maps.append({
            "xqT": qT, "xkT": kT, "xvT": vT,
            "wq": np.ascontiguousarray(Wq[:, cs]),
            "wk": np.ascontiguousarray(Wk[:, cs]),
            "wv": np.ascontiguousarray(Wv[:, cs]),
            "wo": np.ascontiguousarray(Wo[cs, :]),
            "bq": np.ascontiguousarray(bq[cs]),
            "bk": np.ascontiguousarray(bk[cs]),
            "bv": np.ascontiguousarray(bv[cs])[None, :],
            "ones_row": np.ones((1, P), dtype=f32),
            "vones": np.ones((P, (seq // P) * HPC), dtype=f32),
        })
    return in_maps


def combine_outputs(results, bo):
    """Host all-reduce of the Wo row-shard partials + bias."""
    bo = np.asarray(bo, dtype=np.float32)
    outs = []
    for b in range(B):
        acc = results[b * GROUPS]["out"].astype(np.float32).copy()
        for g in range(1, GROUPS):
            acc += results[b * GROUPS + g]["out"]
        outs.append(acc + bo)
    return np.stack(outs)


def kernel(queries, keys, values, Wq, bq, Wk, bk, Wv, bv, Wo, bo):
    nc = _get_program()
    in_maps = make_in_maps(queries, keys, values, Wq, bq, Wk, bk, Wv, bv,
                           Wo, bo)
    res = run_bass_kernel_spmd(nc, in_maps, list(range(NCORES)))
    return combine_outputs(res.results, bo)



# revision 4
# speedup vs baseline: 1.0849x; 1.0849x over previous
"""MultiHeadAttention forward on 8 Trainium2 NeuronCores.

Sharding (Megatron-style tensor parallel x data parallel):
  core c (0..7): batch b = c // 4, head group g = c % 4 (4 of 16 heads).
  Wq/Wk/Wv column-sharded ([1024, 256] per core), Wo row-sharded
  ([256, 1024] per core). Each core computes a partial output
  [S, D] = attn(heads g) @ Wo_rows; the host sums the 4 partials per
  batch and adds bo (the "all-reduce" runs on host since full outputs
  are gathered anyway).

Device dataflow per core (all matmuls in float32r, full PE rate):
  QT/KT [dk-on-partition, S] via lhsT=W chunks, rhs=x^T chunks
  V natural [S-on-partition, 256] (+bias via ones-row matmul,
    +ones column appended for the softmax denominator)
  scores^T[k, q] per (head, q-block): lhsT=KT chunk, rhs=QT block
  E = exp(scores^T / sqrt(S)) via ACT, PSUM->SBUF
  attn^T [65, q] = accumulate lhsT=[V_h|1], rhs=E chunks
    (row 64 = softmax denominator; normalize by its reciprocal)
  O [q, 1024] = lhsT=attn^T chunks, rhs=Wo chunks; DMA PSUM->DRAM
"""

import math

import numpy as np

import concourse.bass as bass
import concourse.mybir as mybir
import concourse.tile as tile
from concourse import bacc
from concourse.bass_utils import run_bass_kernel_spmd

P = 128
B, S, D, H = 2, 2048, 1024, 16
NCORES = 8
GROUPS = NCORES // B          # 4 head-groups
HPC = H // GROUPS             # 4 heads per core
DK = D // H                   # 64
CPC = HPC * DK                # 256 cols per core
NP = CPC // P                 # 2 head pairs per core
DC = D // P                   # 8 contraction chunks over D
QB = 512                      # q block (matmul moving free dim)

F32 = mybir.dt.float32
F32R = mybir.dt.float32r
BF16 = mybir.dt.bfloat16

# DVE rejects partition-step-0 reads, so the normalize uses a
# matmul-based partition broadcast of the reciprocal row.
USE_PARTITION_BCAST = False


def build_program(seq=S):
    KT = seq // P             # k tiles
    NJ = seq // QB            # q blocks
    inv_sqrt_s = 1.0 / math.sqrt(S)  # reference scales by sqrt(full S) always

    nc = bacc.Bacc("TRN2", target_bir_lowering=False, debug=False,
                   num_devices=NCORES)
    xqT = nc.declare_dram_parameter("xqT", [D, seq], BF16, isOutput=False)
    xkT = nc.declare_dram_parameter("xkT", [D, seq], BF16, isOutput=False)
    xvT = nc.declare_dram_parameter("xvT", [D, seq], BF16, isOutput=False)
    wq = nc.declare_dram_parameter("wq", [D, CPC], BF16, isOutput=False)
    wk = nc.declare_dram_parameter("wk", [D, CPC], BF16, isOutput=False)
    wv = nc.declare_dram_parameter("wv", [D, CPC], BF16, isOutput=False)
    wo = nc.declare_dram_parameter("wo", [CPC, D], BF16, isOutput=False)
    bq = nc.declare_dram_parameter("bq", [CPC], F32, isOutput=False)
    bk = nc.declare_dram_parameter("bk", [CPC], F32, isOutput=False)
    bv = nc.declare_dram_parameter("bv", [1, CPC], BF16, isOutput=False)
    # float32r constants (memset can't write f32r: walrus ISA check)
    ones_row = nc.declare_dram_parameter("ones_row", [1, P], BF16,
                                         isOutput=False)
    vones = nc.declare_dram_parameter("vones", [P, KT * HPC], BF16,
                                      isOutput=False)
    out = nc.declare_dram_parameter("out", [seq, D], F32, isOutput=True)

    xqT_r = xqT.rearrange("(dc p) s -> p dc s", p=P)
    xkT_r = xkT.rearrange("(dc p) s -> p dc s", p=P)
    xvT_r = xvT.rearrange("(dc p) s -> p dc s", p=P)

    with tile.TileContext(nc) as tc:
        with tc.tile_pool(name="consts", bufs=1) as consts:
            bq_sb = consts.tile([P, NP], F32)
            bk_sb = consts.tile([P, NP], F32)
            bv_sb = consts.tile([1, CPC], BF16)
            ones_sb = consts.tile([1, P], BF16)
            # weight DMAs are emitted in consumption order (wk with stage-1
            # K, wq with Q, wv before V, wo before the attention pipeline)
            # so the x chunk DMAs are not queued behind cold weights.
            nc.sync.dma_start(bk_sb[:], bk.rearrange("(j p) -> p j", p=P))
            nc.sync.dma_start(bq_sb[:], bq.rearrange("(j p) -> p j", p=P))

            # Persistent activations. Per-pair / per-block tiles keep Tile's
            # dependency tracking fine-grained (stage overlap).
            qt_t = [[consts.tile([P, QB], BF16, name=f"qt_{j}_{qc}")
                     for qc in range(NJ)] for j in range(NP)]
            kt_p = [consts.tile([P, seq], BF16, name=f"kt_p{j}")
                    for j in range(NP)]
            v_sb = consts.tile([P, KT, HPC, DK + 1], BF16)
            at_j = [consts.tile([P, NP, QB], BF16, name=f"at_j{j}")
                    for j in range(NJ)]

            # ---- Stage 1 ----
            def emit_proj(name, x_r, w_src, b_sb, dst_fn, use_dve=None):
                with (
                    tc.tile_pool(name=f"xc_{name}", bufs=6) as xp,
                    tc.tile_pool(name=f"wp_{name}", bufs=1) as wp,
                    tc.tile_pool(name=f"ps_{name}", bufs=2 * NJ,
                                 space="PSUM") as psp,
                ):
                    # per-dc weight tiles, each DMA'd right after its x
                    # chunk: the dc=0 matmuls start after ~1.1MB instead of
                    # waiting for the whole weight matrix.
                    w_r = w_src.rearrange("(dc p) c -> p dc c", p=P)
                    w_dc = [wp.tile([P, CPC], BF16, tag=f"w{dc}",
                                    name=f"w_{name}{dc}")
                            for dc in range(DC)]
                    psq = [psp.tile([P, QB], F32, tag="qk", name=f"psq_{i}")
                           for i in range(NP * NJ)]
                    for dc in range(DC):
                        xt = xp.tile([P, seq], BF16, tag="xc")
                        nc.sync.dma_start(xt[:], x_r[:, dc])
                        nc.sync.dma_start(w_dc[dc][:], w_r[:, dc])
                        for j in range(NP):
                            for qc in range(NJ):
                                nc.tensor.matmul(
                                    psq[j * NJ + qc][:],
                                    w_dc[dc][:, j * P:(j + 1) * P],
                                    xt[:, qc * QB:(qc + 1) * QB],
                                    start=(dc == 0), stop=(dc == DC - 1),
                                )
                    for j in range(NP):
                        for qc in range(NJ):
                            # bias-add psum drains: ACT (idle in stage 1)
                            # except pairs routed to DVE to keep the first
                            # exps from queueing behind them on ACT.
                            if use_dve is not None and use_dve(j, qc):
                                nc.vector.tensor_scalar_add(
                                    dst_fn(j, qc),
                                    psq[j * NJ + qc][:],
                                    b_sb[:, j:j + 1],
                                )
                            else:
                                nc.scalar.activation(
                                    dst_fn(j, qc),
                                    psq[j * NJ + qc][:],
                                    mybir.ActivationFunctionType.Identity,
                                    bias=b_sb[:, j:j + 1],
                                )

            # V projection, streamed like K/Q (dc-outer over 16 kt psums,
            # two kt sharing each psum bank) so PE tracks the x_v DMAs.
            # Emitted as a function so it can be placed AFTER the first
            # lookahead scores blocks: the in-order PE queue then has exp
            # feedstock issued ahead of the xv-DMA-paced V matmuls, and ACT
            # works through early softmax blocks during the x_v stream.
            def emit_v_block():
                with (
                    tc.tile_pool(name="xc_v", bufs=6) as xvp,
                    tc.tile_pool(name="wp_v", bufs=1) as wvp,
                    tc.tile_pool(name="ps_v", bufs=KT // 2,
                                 space="PSUM") as psvp,
                ):
                    wv_r = wv.rearrange("(dc p) c -> p dc c", p=P)
                    wv_dc = [wvp.tile([P, CPC], BF16, tag=f"w{dc}",
                                      name=f"w_v{dc}")
                             for dc in range(DC)]
                    psv = [psvp.tile([P, 2, CPC], F32, tag="v",
                                     name=f"psv_{k2}")
                           for k2 in range(KT // 2)]
                    # two kt share a psum bank => one accumulation group per
                    # bank: start only zeroes on the very first write, stop
                    # on the last bias matmul of the pair.
                    for dc in range(DC):
                        xt = xvp.tile([P, seq], BF16, tag="xc")
                        nc.sync.dma_start(xt[:], xvT_r[:, dc])
                        nc.sync.dma_start(wv_dc[dc][:], wv_r[:, dc])
                        for kt in range(KT):
                            nc.tensor.matmul(
                                psv[kt // 2][:, kt % 2],
                                xt[:, kt * P:(kt + 1) * P],
                                wv_dc[dc][:, :],
                                start=(dc == 0 and kt % 2 == 0), stop=False,
                            )
                        if dc == 0:
                            # constants are only needed at the bias/drain
                            # step; emitting them after xv0/wv0 keeps V's
                            # first matmuls off the DMA critical path.
                            nc.sync.dma_start(bv_sb[:], bv[:])
                            nc.sync.dma_start(ones_sb[:], ones_row[:])
                            with nc.allow_non_contiguous_dma(
                                    reason="one-time 32KB ones-column init"):
                                nc.sync.dma_start(
                                    v_sb[:, :, :, DK:DK + 1],
                                    vones.rearrange(
                                        "p (kt h) -> p kt h",
                                        kt=KT, h=HPC)[:, :, :, None],
                                )
                    for k2 in range(KT // 2):
                        for half in range(2):
                            nc.tensor.matmul(  # += ones^T @ bv  (bias add)
                                psv[k2][:, half], ones_sb[:], bv_sb[:],
                                start=False, stop=(half == 1),
                            )
                        for half in range(2):
                            # one strided copy per k-tile: [128, 4, 64] dest
                            # (skipping the ones column)
                            nc.vector.tensor_copy(
                                v_sb[:, 2 * k2 + half, :, 0:DK],
                                psv[k2][:, half].rearrange(
                                    "p (h d) -> p h d", h=HPC),
                            )

            # Emission order K -> V -> Q: V's xv-DMA-paced matmuls fill the
            # PE while x_q is still streaming, and V is complete before the
            # first attnV -- no V-wall inside the attention pipeline.
            emit_proj("k", xkT_r, wk, bk_sb,
                      lambda j, qc: kt_p[j][:, qc * QB:(qc + 1) * QB])
            emit_v_block()
            emit_proj("q", xqT_r, wq, bq_sb,
                      lambda j, qc: qt_t[j][qc][:],
                      use_dve=lambda j, qc: j == 1)

            # ---- Stages 2+3: pipelined attention + output projection ----
            # Per (j, h) block: scores^T + exp; attnV trails LOOKAHEAD
            # blocks. O projection for q-block j is emitted right after its
            # last head's attnV.
            K2 = KT // 2  # two score k-tiles share one psum / exp op
            LOOKAHEAD = 2
            blocks = [(j, h) for j in range(NJ) for h in range(HPC)]

            def make_emit_scores(pool):
                def emit_scores(j, h, e2):
                    hp, hj = h % 2, h // 2
                    prow = slice(hp * DK, (hp + 1) * DK)
                    for k2 in range(K2):
                        pss = pool.tile([P, 2 * QB], F32, tag="s",
                                        name=f"pss_{j}_{h}_{k2}")
                        for half in range(2):
                            kt = 2 * k2 + half
                            nc.tensor.matmul(
                                pss[:, half * QB:(half + 1) * QB],
                                kt_p[hj][prow, kt * P:(kt + 1) * P],
                                qt_t[hj][j][prow, :],
                                start=True, stop=True,
                            )
                        nc.scalar.activation(
                            e2[:, k2], pss[:],
                            mybir.ActivationFunctionType.Exp,
                            scale=inv_sqrt_s,
                        )
                return emit_scores

            with tc.tile_pool(name="epool", bufs=3) as ep:
                pending = []
                with (
                    tc.tile_pool(name="rpool", bufs=4) as rp,
                    tc.tile_pool(name="opool", bufs=4) as op,
                    tc.tile_pool(name="wop", bufs=1) as wop,
                    tc.tile_pool(name="ps_s", bufs=2, space="PSUM") as pss_p,
                    tc.tile_pool(name="ps_a", bufs=1, space="PSUM") as psa_p,
                    tc.tile_pool(name="ps_p", bufs=1, space="PSUM") as psp_p,
                    tc.tile_pool(name="ps_o", bufs=2, space="PSUM") as pso_p,
                ):
                    wo_sb = wop.tile([P, NP, D], BF16)
                    nc.sync.dma_start(wo_sb[:],
                                      wo.rearrange("(dj p) n -> p dj n", p=P))
                    emit_scores = make_emit_scores(pss_p)

                    def emit_attnv(j, h, e2):
                        hp, hj = h % 2, h // 2
                        prow = slice(hp * DK, (hp + 1) * DK)
                        psa = psa_p.tile([P, QB], F32, tag="a",
                                         name=f"psa_{j}_{h}")
                        for kt in range(KT):
                            nc.tensor.matmul(
                                psa[:DK + 1],
                                v_sb[:, kt, h, :],
                                e2[:, kt // 2,
                                   (kt % 2) * QB:(kt % 2 + 1) * QB],
                                start=(kt == 0), stop=(kt == KT - 1),
                            )
                        rc = rp.tile([1, QB], BF16, tag="rc")
                        with nc.allow_low_precision(
                                reason="f32r reciprocal for matmul bcast"):
                            nc.vector.reciprocal(rc[:], psa[DK:DK + 1, :])
                        prc = psp_p.tile([P, QB], F32, tag="p",
                                         name=f"prc_{j}_{h}")
                        nc.tensor.matmul(
                            prc[:DK], ones_sb[:, :DK], rc[:],
                            start=True, stop=True,
                        )
                        atmp = rp.tile([DK, QB], F32, tag="atmp")
                        nc.vector.tensor_copy(atmp[:], psa[:DK])
                        nc.vector.tensor_tensor(
                            at_j[j][prow, hj, :], atmp[:], prc[:DK],
                            mybir.AluOpType.mult,
                        )

                    def emit_oproj(j):
                        for ql in range(QB // P):
                            qt0 = j * (QB // P) + ql
                            o_sb = op.tile([P, D], F32, tag="o_sb")
                            for nh in range(D // QB):
                                pso = pso_p.tile([P, QB], F32, tag="o",
                                                 name=f"pso_{qt0}_{nh}")
                                for dj in range(NP):
                                    nc.tensor.matmul(
                                        pso[:],
                                        at_j[j][:, dj, ql * P:(ql + 1) * P],
                                        wo_sb[:, dj,
                                              nh * QB:(nh + 1) * QB],
                                        start=(dj == 0),
                                        stop=(dj == NP - 1),
                                    )
                                nc.vector.tensor_copy(
                                    o_sb[:, nh * QB:(nh + 1) * QB], pso[:])
                                # flush each half as soon as its copy lands
                                nc.sync.dma_start(
                                    out[qt0 * P:(qt0 + 1) * P,
                                        nh * QB:(nh + 1) * QB],
                                    o_sb[:, nh * QB:(nh + 1) * QB],
                                )

                    for (j, h) in blocks:
                        e2 = ep.tile([P, K2, 2 * QB], BF16, tag="E",
                                     name=f"e2_{j}_{h}")
                        emit_scores(j, h, e2)
                        pending.append((j, h, e2))
                        if len(pending) > LOOKAHEAD:
                            jj, hh, ee = pending.pop(0)
                            emit_attnv(jj, hh, ee)
                            if hh == HPC - 1:
                                emit_oproj(jj)
                    for jj, hh, ee in pending:
                        emit_attnv(jj, hh, ee)
                        if hh == HPC - 1:
                            emit_oproj(jj)

    nc.compile()
    return nc


_PROGRAM_CACHE = {}


def _get_program(seq=S):
    if seq not in _PROGRAM_CACHE:
        _PROGRAM_CACHE[seq] = build_program(seq)
    return _PROGRAM_CACHE[seq]


def make_in_maps(queries, keys, values, Wq, bq, Wk, bk, Wv, bv, Wo, bo):
    """Per-core input dicts implementing the sharding."""
    import ml_dtypes
    f32 = np.float32
    bf16 = ml_dtypes.bfloat16
    seq = np.asarray(queries).shape[1]
    xT = {}
    for b in range(B):
        xT[b] = (
            np.ascontiguousarray(np.asarray(queries[b], dtype=f32).T).astype(bf16),
            np.ascontiguousarray(np.asarray(keys[b], dtype=f32).T).astype(bf16),
            np.ascontiguousarray(np.asarray(values[b], dtype=f32).T).astype(bf16),
        )
    Wq, Wk, Wv, Wo = (np.asarray(a, dtype=f32) for a in (Wq, Wk, Wv, Wo))
    bq, bk, bv = (np.asarray(a, dtype=f32) for a in (bq, bk, bv))
    in_maps = []
    for c in range(NCORES):
        b, g = divmod(c, GROUPS)
        cs = slice(g * CPC, (g + 1) * CPC)
        qT, kT, vT = xT[b]
        in_maps.append({
            "xqT": qT, "xkT": kT, "xvT": vT,
            "wq": np.ascontiguousarray(Wq[:, cs]).astype(bf16),
            "wk": np.ascontiguousarray(Wk[:, cs]).astype(bf16),
            "wv": np.ascontiguousarray(Wv[:, cs]).astype(bf16),
            "wo": np.ascontiguousarray(Wo[cs, :]).astype(bf16),
            "bq": np.ascontiguousarray(bq[cs]),
            "bk": np.ascontiguousarray(bk[cs]),
            "bv": np.ascontiguousarray(bv[cs])[None, :].astype(bf16),
            "ones_row": np.ones((1, P), dtype=bf16),
            "vones": np.ones((P, (seq // P) * HPC), dtype=bf16),
        })
    return in_maps


def combine_outputs(results, bo):
    """Host all-reduce of the Wo row-shard partials + bias."""
    bo = np.asarray(bo, dtype=np.float32)
    outs = []
    for b in range(B):
        acc = results[b * GROUPS]["out"].astype(np.float32).copy()
        for g in range(1, GROUPS):
            acc += results[b * GROUPS + g]["out"]
        outs.append(acc + bo)
    return np.stack(outs)


def kernel(queries, keys, values, Wq, bq, Wk, bk, Wv, bv, Wo, bo):
    nc = _get_program()
    in_maps = make_in_maps(queries, keys, values, Wq, bq, Wk, bk, Wv, bv,
                           Wo, bo)
    res = run_bass_kernel_spmd(nc, in_maps, list(range(NCORES)))
    return combine_outputs(res.results, bo)



# revision 7
# speedup vs baseline: 1.3370x; 1.2324x over previous
"""MultiHeadAttention forward on 8 Trainium2 NeuronCores (bf16, v3).

Sharding (Megatron-style tensor parallel x data parallel):
  core c (0..7): batch b = c // 4, head group g = c % 4 (4 of 16 heads).
  Wq/Wk/Wv column-sharded ([1024, 256] per core), Wo row-sharded
  ([256, 1024] per core). Each core computes a partial output
  [S, D] = attn(heads g) @ Wo_rows; the host sums the 4 partials per
  batch and adds bo.

The kernel is ACT(exp)-bound: 16.8M exps at 128/cycle/1.2GHz = 133us.
Everything else is scheduled to (a) start the exp stream as early as
the xk+xq DMA allows (~28us) and (b) never starve it:
  - bf16 everywhere (DMA halved, matmuls full rate)
  - warm-up matmuls on zeros ramp the PE p-state before xk lands
  - DMA order xk, xq, xv; K is chunk-streamed, Q is chunk-resident and
    computed q-block-0-first so scores start right after xq lands
  - V projection in 4 passes of 4 k-tiles on 2 PSUM banks, emitted
    consecutively after the first 4 scores blocks (its PE time hides
    under the ACT-paced pipeline)
  - attnV flipped to [q-part, dk+1] (lhsT = E tile): half the PE rows,
    normalize is reciprocal + per-partition tensor_scalar on DVE, and a
    PE transpose rebuilds [c, q] for the O projection
PSUM banks (left|right): warm1|0 -> K8|0 -> pss4|psq2 -> pss4|psq2+
psv2 -> pss4+psa1+ptr1|psq2 -> pss4+psa1+ptr1+pso2|0.
"""

import math

import numpy as np

import concourse.bass as bass
import concourse.mybir as mybir
import concourse.tile as tile
from concourse import bacc
from concourse.bass_utils import run_bass_kernel_spmd

P = 128
B, S, D, H = 2, 2048, 1024, 16
NCORES = 8
GROUPS = NCORES // B          # 4 head-groups
HPC = H // GROUPS             # 4 heads per core
DK = D // H                   # 64
CPC = HPC * DK                # 256 cols per core
NP = CPC // P                 # 2 head pairs per core
DC = D // P                   # 8 contraction chunks over D
QB = 512                      # q block (matmul moving free dim)
QT = QB // P                  # 4 q-subtiles per block

WARMUP = 10                   # p-state ramp matmuls on zeros
LOOKAHEAD = 3                 # scored blocks in flight before attnV

F32 = mybir.dt.float32
BF16 = mybir.dt.bfloat16
Act = mybir.ActivationFunctionType


def build_program(seq=S):
    KT = seq // P             # 16 k tiles
    NJ = seq // QB            # 4 q blocks
    K2 = KT // 2
    inv_sqrt_s = 1.0 / math.sqrt(S)

    nc = bacc.Bacc("TRN2", target_bir_lowering=False, debug=False,
                   num_devices=NCORES)
    xqT = nc.declare_dram_parameter("xqT", [D, seq], BF16, isOutput=False)
    xkT = nc.declare_dram_parameter("xkT", [D, seq], BF16, isOutput=False)
    xvT = nc.declare_dram_parameter("xvT", [D, seq], BF16, isOutput=False)
    wq = nc.declare_dram_parameter("wq", [D, CPC], BF16, isOutput=False)
    wk = nc.declare_dram_parameter("wk", [D, CPC], BF16, isOutput=False)
    wv = nc.declare_dram_parameter("wv", [D, CPC], BF16, isOutput=False)
    wo = nc.declare_dram_parameter("wo", [CPC, D], BF16, isOutput=False)
    bq = nc.declare_dram_parameter("bq", [CPC], F32, isOutput=False)
    bk = nc.declare_dram_parameter("bk", [CPC], F32, isOutput=False)
    bv = nc.declare_dram_parameter("bv", [1, CPC], BF16, isOutput=False)
    ones_row = nc.declare_dram_parameter("ones_row", [1, P], BF16,
                                         isOutput=False)
    vones = nc.declare_dram_parameter("vones", [P, KT * HPC], BF16,
                                      isOutput=False)
    ident = nc.declare_dram_parameter("ident", [P, P], F32, isOutput=False)
    out = nc.declare_dram_parameter("out", [seq, D], F32, isOutput=True)

    xqT_r = xqT.rearrange("(dc p) s -> p dc s", p=P)
    xkT_r = xkT.rearrange("(dc p) s -> p dc s", p=P)
    xvT_r = xvT.rearrange("(dc p) s -> p dc s", p=P)

    with tile.TileContext(nc) as tc:
        with tc.tile_pool(name="consts", bufs=1) as consts:
            bq_sb = consts.tile([P, NP], F32)
            bk_sb = consts.tile([P, NP], F32)
            bv_sb = consts.tile([1, CPC], BF16)
            ident_sb = consts.tile([P, P], F32)
            ones_sb = consts.tile([1, P], BF16)
            scratch = consts.tile([P, 1], F32)
            zw = consts.tile([P, P], BF16)
            zx = consts.tile([P, QB], BF16)

            # Persistent activations
            qt_t = [[consts.tile([P, QB], BF16, name=f"qt_{j}_{qc}")
                     for qc in range(NJ)] for j in range(NP)]
            kt_p = [consts.tile([P, seq], BF16, name=f"kt_p{j}")
                    for j in range(NP)]
            v_sb = consts.tile([P, KT, HPC, DK + 1], BF16)
            at_j = [consts.tile([P, NP, QB], BF16, name=f"at_j{j}")
                    for j in range(NJ)]
            # softmax-denominator ones column of V
            nc.vector.memset(v_sb[:, :, :, DK:DK + 1], 1.0)


            # ---- K projection (xk-streamed, 8 psum banks)
            with (
                tc.tile_pool(name="xc_k", bufs=6) as xkp,
                tc.tile_pool(name="wp_k", bufs=1) as wkp,
                tc.tile_pool(name="ps_k", bufs=1, space="PSUM") as pskp,
            ):
                wk_r = wk.rearrange("(dc p) c -> p dc c", p=P)
                wk_dc = [wkp.tile([P, CPC], BF16, tag=f"w{dc}",
                                  name=f"w_k{dc}") for dc in range(DC)]
                psk = [pskp.tile([P, NJ, QB], F32, tag=f"qk{j}",
                                 name=f"psk_{j}")
                       for j in range(NP)]
                xk_t = []
                for dc in range(DC):
                    xt = xkp.tile([P, seq], BF16, tag="xc")
                    nc.sync.dma_start(xt[:], xkT_r[:, dc])
                    nc.sync.dma_start(wk_dc[dc][:], wk_r[:, dc])
                    xk_t.append(xt)
                    if dc == 0:
                        # small consts ride the queue behind the first chunk
                        nc.sync.dma_start(
                            bk_sb[:], bk.rearrange("(j p) -> p j", p=P))
                        nc.sync.dma_start(
                            bq_sb[:], bq.rearrange("(j p) -> p j", p=P))
                        nc.sync.dma_start(ident_sb[:], ident[:])
                        # dummy exp: preloads the ACT function table
                        nc.scalar.activation(scratch[:], bk_sb[:, 0:1],
                                             Act.Exp)
                    for j in range(NP):
                        for qc in range(NJ):
                            nc.tensor.matmul(
                                psk[j][:, qc],
                                wk_dc[dc][:, j * P:(j + 1) * P],
                                xt[:, qc * QB:(qc + 1) * QB],
                                start=(dc == 0), stop=(dc == DC - 1),
                            )
                # xq DMAs queue right behind xk; q-block-0's columns are
                # split out and land first so scores start ~10us earlier
                xq_pool = tc.tile_pool(name="xc_q", bufs=1, side="right")
                wq_pool = tc.tile_pool(name="wp_q", bufs=1, side="right")
                xqp = xq_pool.__enter__()
                wqp = wq_pool.__enter__()
                wq_r = wq.rearrange("(dc p) c -> p dc c", p=P)
                # separate tiles per DMA piece: Tile tracks deps per tile,
                # so q0's early dc-chunks become usable as they land
                wq_t = [wqp.tile([P, 4, CPC], BF16, name=f"w_q{i}")
                        for i in range(2)]
                xq0_t = [xqp.tile([P, 2, QB], BF16, name=f"x_q0_{i}")
                         for i in range(4)]
                nc.sync.dma_start(wq_t[0][:], wq_r[:, 0:4])
                nc.sync.dma_start(xq0_t[0][:], xqT_r[:, 0:2, 0:QB])
                nc.sync.dma_start(xq0_t[1][:], xqT_r[:, 2:4, 0:QB])
                nc.sync.dma_start(wq_t[1][:], wq_r[:, 4:8])
                nc.sync.dma_start(xq0_t[2][:], xqT_r[:, 4:6, 0:QB])
                nc.sync.dma_start(xq0_t[3][:], xqT_r[:, 6:8, 0:QB])
                nc.sync.dma_start(bv_sb[:], bv[:])
                nc.sync.dma_start(ones_sb[:], ones_row[:])
                xqr_sb = xqp.tile([P, DC, (NJ - 1) * QB], BF16, name="x_qr")
                nc.sync.dma_start(xqr_sb[:], xqT_r[:, :, QB:seq])
                # two big K drains (one per head-pair) so the psum banks
                # release fast and the psq pool can open early
                nc.scalar.activation(
                    kt_p[0][:], psk[0].rearrange("p q s -> p (q s)"),
                    Act.Identity, bias=bk_sb[:, 0:1],
                )
                nc.vector.tensor_scalar_add(
                    kt_p[1][:], psk[1].rearrange("p q s -> p (q s)"),
                    bk_sb[:, 1:2],
                )

            # ---- long-lived pipeline pools
            pss_pool = tc.tile_pool(name="ps_s", bufs=2, space="PSUM")
            pssp = pss_pool.__enter__()
            ep_pool = tc.tile_pool(name="epool", bufs=6)
            ep = ep_pool.__enter__()
            psq_pool = tc.tile_pool(name="ps_q", bufs=2, space="PSUM",
                                    side="right")
            psqp = psq_pool.__enter__()

            def emit_q(qc, first):
                psq = [psqp.tile([P, QB], F32, tag="q", name=f"psq_{qc}_{j}")
                       for j in range(NP)]
                jorder = [(j, dc) for j in range(NP) for dc in range(DC)]
                if qc != 0:
                    jorder = [(j, dc) for dc in range(DC) for j in range(NP)]
                for j, dc in jorder:
                    rhs = (xq0_t[dc // 2][:, dc % 2] if qc == 0 else
                           xqr_sb[:, dc, (qc - 1) * QB:qc * QB])
                    nc.tensor.matmul(
                        psq[j][:],
                        wq_t[dc // 4][:, dc % 4, j * P:(j + 1) * P],
                        rhs,
                        start=(dc == 0), stop=(dc == DC - 1),
                    )
                for j in range(NP):
                    if first and j == 0:
                        nc.scalar.activation(
                            qt_t[j][qc][:], psq[j][:],
                            Act.Identity, bias=bq_sb[:, j:j + 1],
                        )
                    else:
                        nc.vector.tensor_scalar_add(
                            qt_t[j][qc][:], psq[j][:], bq_sb[:, j:j + 1],
                        )

            def emit_scores(j, h, filler=None):
                hp, hj = h % 2, h // 2
                prow = slice(hp * DK, (hp + 1) * DK)
                e2 = ep.tile([P, K2, 2 * QB], BF16, tag="E",
                             name=f"e2_{j}_{h}")
                for k2 in range(K2):
                    pss = pssp.tile([P, 2 * QB], F32, tag="s",
                                    name=f"pss_{j}_{h}_{k2}")
                    for half in range(2):
                        kt = 2 * k2 + half
                        nc.tensor.matmul(
                            pss[:, half * QB:(half + 1) * QB],
                            kt_p[hj][prow, kt * P:(kt + 1) * P],
                            qt_t[hj][j][prow, :],
                            start=True, stop=True,
                        )
                    nc.scalar.activation(
                        e2[:, k2], pss[:], Act.Exp, scale=inv_sqrt_s,
                    )
                    if filler is not None:
                        filler()
                return e2

            # ---- emission: Q early, scores ASAP, V woven into S1..S3
            # at scores-group granularity (PE slack hides V's 15us under
            # the ACT-paced exp stream without blocking it)
            e2_of = {}
            blocks = [(j, h) for j in range(NJ) for h in range(HPC)]
            pending = []

            emit_q(0, first=True)
            j0, h0 = blocks[0]
            e2_of[(j0, h0)] = emit_scores(j0, h0)
            pending.append((j0, h0))
            with tc.high_priority(offset=-1_000_000):
                emit_q(1, first=False)

            # V pools + DMAs (xv queues right behind xq)
            xv_pool = tc.tile_pool(name="xc_v", bufs=1, side="right")
            wv_pool = tc.tile_pool(name="wp_v", bufs=1, side="right")
            psv_pool = tc.tile_pool(name="ps_v", bufs=2, space="PSUM",
                                    side="right")
            xvp = xv_pool.__enter__()
            wvp = wv_pool.__enter__()
            psvp = psv_pool.__enter__()
            wv_r = wv.rearrange("(dc p) c -> p dc c", p=P)
            xv_dc, wv_dc = [], []
            for dc in range(DC):
                xt = xvp.tile([P, seq], BF16, tag=f"x{dc}", name=f"x_v{dc}")
                nc.sync.dma_start(xt[:], xvT_r[:, dc])
                xv_dc.append(xt)
                wt = wvp.tile([P, CPC], BF16, tag=f"w{dc}", name=f"w_v{dc}")
                nc.sync.dma_start(wt[:], wv_r[:, dc])
                wv_dc.append(wt)

            # V piece list: ("mm", vp, dc) x32 + ("fin", vp) x4
            v_pieces = []
            for vp in range(4):
                v_pieces.extend(("mm", vp, dc) for dc in range(DC))
                v_pieces.append(("fin", vp))
            psv_tiles = {}
            v_idx = [0]
            n_groups = 3 * K2  # filler spread over S1..S3

            def emit_v_piece(kind, vp, dc=0):
                if kind == "mm":
                    if dc == 0:
                        psv_tiles[vp] = [
                            psvp.tile([P, 2, CPC], F32, tag="v",
                                      name=f"psv_{vp}_{i}")
                            for i in range(2)]
                    psv = psv_tiles[vp]
                    for kk in range(4):
                        kt = 4 * vp + kk
                        nc.tensor.matmul(
                            psv[kk // 2][:, kk % 2],
                            xv_dc[dc][:, kt * P:(kt + 1) * P],
                            wv_dc[dc][:, :],
                            start=(dc == 0 and kk % 2 == 0), stop=False,
                        )
                else:
                    psv = psv_tiles[vp]
                    for kk in range(4):
                        nc.tensor.matmul(  # += ones^T @ bv  (bias add)
                            psv[kk // 2][:, kk % 2], ones_sb[:], bv_sb[:],
                            start=False, stop=(kk % 2 == 1),
                        )
                    for kk in range(4):
                        kt = 4 * vp + kk
                        nc.vector.tensor_copy(
                            v_sb[:, kt, :, 0:DK],
                            psv[kk // 2][:, kk % 2].rearrange(
                                "p (h d) -> p h d", h=HPC),
                        )

            group_no = [0]

            def v_filler():
                group_no[0] += 1
                target = (len(v_pieces) * group_no[0]) // n_groups
                with tc.high_priority(offset=-400):
                    while v_idx[0] < min(target, len(v_pieces)):
                        emit_v_piece(*v_pieces[v_idx[0]])
                        v_idx[0] += 1

            for b in (1, 2, 3):
                j, h = blocks[b]
                e2_of[(j, h)] = emit_scores(j, h, filler=v_filler)
                pending.append((j, h))
            with tc.high_priority(offset=-400):
                while v_idx[0] < len(v_pieces):  # finish any leftovers
                    emit_v_piece(*v_pieces[v_idx[0]])
                    v_idx[0] += 1
            psv_pool.__exit__(None, None, None)
            wv_pool.__exit__(None, None, None)
            xv_pool.__exit__(None, None, None)

            def emit_s(b):
                j, h = blocks[b]
                e2_of[(j, h)] = emit_scores(j, h)
                pending.append((j, h))

            # ---- flipped attnV + pair transpose pipeline
            psa_pool = tc.tile_pool(name="ps_a", bufs=1, space="PSUM")
            ptr_pool = tc.tile_pool(name="ps_t", bufs=1, space="PSUM")
            asb_pool = tc.tile_pool(name="asb", bufs=8)
            rc_pool = tc.tile_pool(name="rpool", bufs=4)
            psap = psa_pool.__enter__()
            ptrp = ptr_pool.__enter__()
            asbp = asb_pool.__enter__()
            rcp = rc_pool.__enter__()
            psop = op = wo_sb = None

            def emit_attnv(j, h, pair, half):
                e2 = e2_of.pop((j, h))
                psa = psap.tile([P, QT, DK + 4], F32, tag="a",
                                name=f"psa_{j}_{h}")
                for kt in range(KT):
                    for g in range(QT):
                        q0 = (kt % 2) * QB + g * P
                        nc.tensor.matmul(
                            psa[:, g, 0:DK + 1],
                            e2[:, kt // 2, q0:q0 + P],
                            v_sb[:, kt, h, :],
                            start=(g == 0 and kt == 0),
                            stop=(g == QT - 1 and kt == KT - 1),
                        )
                for g in range(QT):
                    rc = rcp.tile([P, 1], F32, tag="rc")
                    nc.vector.reciprocal(rc[:], psa[:, g, DK:DK + 1])
                    nc.vector.tensor_scalar_mul(
                        pair[g][:, half], psa[:, g, 0:DK], rc[:])

            def emit_transpose_g(j, hj, pair, g):
                ptr = ptrp.tile([P, P], F32, tag="t",
                                name=f"ptr_{j}_{hj}_{g}")
                nc.tensor.matmul(
                    ptr[:], pair[g].rearrange("p h d -> p (h d)"),
                    ident_sb[:], is_transpose=True,
                    start=True, stop=True,
                )
                nc.vector.tensor_copy(
                    at_j[j][:, hj, g * P:(g + 1) * P], ptr[:])

            def emit_oproj_ql(j, ql):
                qt0 = j * QT + ql
                o_sb = op.tile([P, D], F32, tag="o_sb")
                for nh in range(D // QB):
                    pso = psop.tile([P, QB], F32, tag="o",
                                    name=f"pso_{qt0}_{nh}")
                    for dj in range(NP):
                        nc.tensor.matmul(
                            pso[:],
                            at_j[j][:, dj, ql * P:(ql + 1) * P],
                            wo_sb[:, dj, nh * QB:(nh + 1) * QB],
                            start=(dj == 0), stop=(dj == NP - 1),
                        )
                    nc.vector.tensor_copy(
                        o_sb[:, nh * QB:(nh + 1) * QB], pso[:])
                    nc.sync.dma_start(
                        out[qt0 * P:(qt0 + 1) * P,
                            nh * QB:(nh + 1) * QB],
                        o_sb[:, nh * QB:(nh + 1) * QB],
                    )

            deferred_o = []
            asb_h0 = {}

            def pop_one():
                jj, hh = pending.pop(0)
                hj = hh // 2
                if hh % 2 == 0:
                    pair = [asbp.tile([P, 2, DK], F32, tag="asb",
                                      name=f"asb_{jj}_{hj}_{g}")
                            for g in range(QT)]
                    asb_h0[hj] = pair
                    emit_attnv(jj, hh, pair, 0)
                    return
                pair = asb_h0.pop(hj)
                emit_attnv(jj, hh, pair, 1)
                if hh == HPC - 1:
                    for g in range(QT):
                        emit_transpose_g(jj, hj, pair, g)
                        if psop is None:
                            deferred_o.append(jj)
                        else:
                            emit_oproj_ql(jj, g)
                else:
                    for g in range(QT):
                        emit_transpose_g(jj, hj, pair, g)

            pop_one()  # A(S0)
            pop_one()  # A(S1)
            for b in range(4, len(blocks)):
                if b == 8:
                    with tc.high_priority(offset=-1_000_000):
                        emit_q(2, first=False)
                if b == 9:
                    with tc.high_priority(offset=-1_000_000):
                        emit_q(3, first=False)
                    psq_pool.__exit__(None, None, None)
                    wq_pool.__exit__(None, None, None)
                    xq_pool.__exit__(None, None, None)
                    # O-projection pools (they need psq's 2 banks)
                    pso_pool = tc.tile_pool(name="ps_o", bufs=2,
                                            space="PSUM")
                    op_pool = tc.tile_pool(name="opool", bufs=2)
                    wo_pool = tc.tile_pool(name="wop", bufs=1)
                    psop = pso_pool.__enter__()
                    op = op_pool.__enter__()
                    wop = wo_pool.__enter__()
                    wo_sb = wop.tile([P, NP, D], BF16)
                    nc.sync.dma_start(
                        wo_sb[:], wo.rearrange("(dj p) n -> p dj n", p=P))
                    with tc.high_priority(offset=-400):
                        for jd in sorted(set(deferred_o)):
                            for g in range(QT):
                                emit_oproj_ql(jd, g)
                    deferred_o.clear()
                emit_s(b)
                while len(pending) > LOOKAHEAD:
                    pop_one()
            while pending:
                pop_one()

            for pool in (wo_pool, op_pool, pso_pool, rc_pool, asb_pool,
                         ptr_pool, psa_pool, ep_pool, pss_pool):
                pool.__exit__(None, None, None)

    nc.compile()
    return nc


_PROGRAM_CACHE = {}


def _get_program(seq=S):
    if seq not in _PROGRAM_CACHE:
        _PROGRAM_CACHE[seq] = build_program(seq)
    return _PROGRAM_CACHE[seq]


def make_in_maps(queries, keys, values, Wq, bq, Wk, bk, Wv, bv, Wo, bo):
    """Per-core input dicts implementing the sharding."""
    import ml_dtypes
    f32 = np.float32
    bf16 = ml_dtypes.bfloat16
    xT = {}
    for b in range(B):
        xT[b] = (
            np.ascontiguousarray(
                np.asarray(queries[b], dtype=f32).T).astype(bf16),
            np.ascontiguousarray(
                np.asarray(keys[b], dtype=f32).T).astype(bf16),
            np.ascontiguousarray(
                np.asarray(values[b], dtype=f32).T).astype(bf16),
        )
    Wq, Wk, Wv, Wo = (np.asarray(a, dtype=f32) for a in (Wq, Wk, Wv, Wo))
    bq, bk, bv = (np.asarray(a, dtype=f32) for a in (bq, bk, bv))
    in_maps = []
    for c in range(NCORES):
        b, g = divmod(c, GROUPS)
        cs = slice(g * CPC, (g + 1) * CPC)
        qT, kT, vT = xT[b]
        in_maps.append({
            "xqT": qT, "xkT": kT, "xvT": vT,
            "wq": np.ascontiguousarray(Wq[:, cs]).astype(bf16),
            "wk": np.ascontiguousarray(Wk[:, cs]).astype(bf16),
            "wv": np.ascontiguousarray(Wv[:, cs]).astype(bf16),
            "wo": np.ascontiguousarray(Wo[cs, :]).astype(bf16),
            "bq": np.ascontiguousarray(bq[cs]),
            "bk": np.ascontiguousarray(bk[cs]),
            "bv": np.ascontiguousarray(bv[cs])[None, :].astype(bf16),
            "ones_row": np.ones((1, P), dtype=bf16),
            "ident": np.eye(P, dtype=f32),
            "vones": np.ones((P, 16 * HPC), dtype=bf16),
        })
    return in_maps


def combine_outputs(results, bo):
    """Host all-reduce of the Wo row-shard partials + bias."""
    bo = np.asarray(bo, dtype=np.float32)
    outs = []
    for b in range(B):
        acc = results[b * GROUPS]["out"].astype(np.float32)
        for g in range(1, GROUPS):
            acc = acc + results[b * GROUPS + g]["out"].astype(np.float32)
        outs.append(acc + bo)
    return np.stack(outs)


def kernel(queries, keys, values, Wq, bq, Wk, bk, Wv, bv, Wo, bo):
    nc = _get_program()
    in_maps = make_in_maps(queries, keys, values, Wq, bq, Wk, bk, Wv, bv,
                           Wo, bo)
    res = run_bass_kernel_spmd(nc, in_maps, list(range(NCORES)))
    return combine_outputs(res.results, bo)


# revision 8
# speedup vs baseline: 1.3563x; 1.0145x over previous
"""MultiHeadAttention forward on 8 Trainium2 NeuronCores (bf16, v3).

Sharding (Megatron-style tensor parallel x data parallel):
  core c (0..7): batch b = c // 4, head group g = c % 4 (4 of 16 heads).
  Wq/Wk/Wv column-sharded ([1024, 256] per core), Wo row-sharded
  ([256, 1024] per core). Each core computes a partial output
  [S, D] = attn(heads g) @ Wo_rows; the host sums the 4 partials per
  batch and adds bo.

The kernel is ACT(exp)-bound: 16.8M exps at 128/cycle/1.2GHz = 133us.
Everything else is scheduled to (a) start the exp stream as early as
the xk+xq DMA allows (~28us) and (b) never starve it:
  - bf16 everywhere (DMA halved, matmuls full rate)
  - warm-up matmuls on zeros ramp the PE p-state before xk lands
  - DMA order xk, xq, xv; K is chunk-streamed, Q is chunk-resident and
    computed q-block-0-first so scores start right after xq lands
  - V projection in 4 passes of 4 k-tiles on 2 PSUM banks, emitted
    consecutively after the first 4 scores blocks (its PE time hides
    under the ACT-paced pipeline)
  - attnV flipped to [q-part, dk+1] (lhsT = E tile): half the PE rows,
    normalize is reciprocal + per-partition tensor_scalar on DVE, and a
    PE transpose rebuilds [c, q] for the O projection
PSUM banks (left|right): warm1|0 -> K8|0 -> pss4|psq2 -> pss4|psq2+
psv2 -> pss4+psa1+ptr1|psq2 -> pss4+psa1+ptr1+pso2|0.
"""

import math

import numpy as np

import concourse.bass as bass
import concourse.mybir as mybir
import concourse.tile as tile
from concourse import bacc
from concourse.bass_utils import run_bass_kernel_spmd

P = 128
B, S, D, H = 2, 2048, 1024, 16
NCORES = 8
GROUPS = NCORES // B          # 4 head-groups
HPC = H // GROUPS             # 4 heads per core
DK = D // H                   # 64
CPC = HPC * DK                # 256 cols per core
NP = CPC // P                 # 2 head pairs per core
DC = D // P                   # 8 contraction chunks over D
QB = 512                      # q block (matmul moving free dim)
QT = QB // P                  # 4 q-subtiles per block

WARMUP = 10                   # p-state ramp matmuls on zeros
LOOKAHEAD = 3                 # scored blocks in flight before attnV

F32 = mybir.dt.float32
BF16 = mybir.dt.bfloat16
Act = mybir.ActivationFunctionType


def build_program(seq=S):
    KT = seq // P             # 16 k tiles
    NJ = seq // QB            # 4 q blocks
    K2 = KT // 2
    inv_sqrt_s = 1.0 / math.sqrt(S)

    nc = bacc.Bacc("TRN2", target_bir_lowering=False, debug=False,
                   num_devices=NCORES)
    xqT = nc.declare_dram_parameter("xqT", [D, seq], BF16, isOutput=False)
    xkT = nc.declare_dram_parameter("xkT", [D, seq], BF16, isOutput=False)
    xvT = nc.declare_dram_parameter("xvT", [D, seq], BF16, isOutput=False)
    wq = nc.declare_dram_parameter("wq", [D, CPC], BF16, isOutput=False)
    wk = nc.declare_dram_parameter("wk", [D, CPC], BF16, isOutput=False)
    wv = nc.declare_dram_parameter("wv", [D, CPC], BF16, isOutput=False)
    wo = nc.declare_dram_parameter("wo", [CPC, D], BF16, isOutput=False)
    bq = nc.declare_dram_parameter("bq", [CPC], F32, isOutput=False)
    bk = nc.declare_dram_parameter("bk", [CPC], F32, isOutput=False)
    bv = nc.declare_dram_parameter("bv", [CPC], F32, isOutput=False)
    ident = nc.declare_dram_parameter("ident", [P, P], F32, isOutput=False)
    vones = nc.declare_dram_parameter("vones", [P, KT * HPC], BF16,
                                      isOutput=False)
    out = nc.declare_dram_parameter("out", [seq, D], F32, isOutput=True)

    xqT_r = xqT.rearrange("(dc p) s -> p dc s", p=P)
    xkT_r = xkT.rearrange("(dc p) s -> p dc s", p=P)
    xvT_r = xvT.rearrange("(dc p) s -> p dc s", p=P)

    with tile.TileContext(nc) as tc:
        with tc.tile_pool(name="consts", bufs=1) as consts:
            bq_sb = consts.tile([P, NP], F32)
            bk_sb = consts.tile([P, NP], F32)
            bv_c = consts.tile([P, NP], F32)
            ident_sb = consts.tile([P, P], F32)
            scratch = consts.tile([P, 1], F32)
            zw = consts.tile([P, P], BF16)
            zx = consts.tile([P, QB], BF16)

            # Persistent activations
            qt_t = [[consts.tile([P, QB], BF16, name=f"qt_{j}_{qc}")
                     for qc in range(NJ)] for j in range(NP)]
            kt_p = [consts.tile([P, seq], BF16, name=f"kt_p{j}")
                    for j in range(NP)]
            v_sb = consts.tile([P, KT, HPC, DK + 1], BF16)
            at_j = [consts.tile([P, NP, QB], BF16, name=f"at_j{j}")
                    for j in range(NJ)]
            # softmax-denominator ones column of V
            nc.vector.memset(v_sb[:, :, :, DK:DK + 1], 1.0)


            # ---- K projection (xk-streamed, 8 psum banks)
            with (
                tc.tile_pool(name="xc_k", bufs=6) as xkp,
                tc.tile_pool(name="wp_k", bufs=1) as wkp,
                tc.tile_pool(name="ps_k", bufs=1, space="PSUM") as pskp,
            ):
                wk_r = wk.rearrange("(dc p) c -> p dc c", p=P)
                wk_dc = [wkp.tile([P, CPC], BF16, tag=f"w{dc}",
                                  name=f"w_k{dc}") for dc in range(DC)]
                psk = [pskp.tile([P, NJ, QB], F32, tag=f"qk{j}",
                                 name=f"psk_{j}")
                       for j in range(NP)]
                xk_t = []
                for dc in range(DC):
                    xt = xkp.tile([P, seq], BF16, tag="xc")
                    nc.sync.dma_start(xt[:], xkT_r[:, dc])
                    nc.sync.dma_start(wk_dc[dc][:], wk_r[:, dc])
                    xk_t.append(xt)
                    if dc == 0:
                        # small consts ride the queue behind the first chunk
                        nc.sync.dma_start(
                            bk_sb[:], bk.rearrange("(j p) -> p j", p=P))
                        nc.sync.dma_start(
                            bq_sb[:], bq.rearrange("(j p) -> p j", p=P))
                        nc.sync.dma_start(ident_sb[:], ident[:])
                        # dummy exp: preloads the ACT function table
                        nc.scalar.activation(scratch[:], bk_sb[:, 0:1],
                                             Act.Exp)
                    for j in range(NP):
                        for qc in range(NJ):
                            nc.tensor.matmul(
                                psk[j][:, qc],
                                wk_dc[dc][:, j * P:(j + 1) * P],
                                xt[:, qc * QB:(qc + 1) * QB],
                                start=(dc == 0), stop=(dc == DC - 1),
                            )
                # xq DMAs queue right behind xk; q-block-0's columns are
                # split out and land first so scores start ~10us earlier
                xq_pool = tc.tile_pool(name="xc_q", bufs=1, side="right")
                wq_pool = tc.tile_pool(name="wp_q", bufs=1, side="right")
                xqp = xq_pool.__enter__()
                wqp = wq_pool.__enter__()
                wq_r = wq.rearrange("(dc p) c -> p dc c", p=P)
                # separate tiles per DMA piece: Tile tracks deps per tile,
                # so q0's early dc-chunks become usable as they land
                wq_t = [wqp.tile([P, 4, CPC], BF16, name=f"w_q{i}")
                        for i in range(2)]
                xq0_t = [xqp.tile([P, 2, QB], BF16, name=f"x_q0_{i}")
                         for i in range(4)]
                nc.sync.dma_start(wq_t[0][:], wq_r[:, 0:4])
                nc.sync.dma_start(xq0_t[0][:], xqT_r[:, 0:2, 0:QB])
                nc.sync.dma_start(xq0_t[1][:], xqT_r[:, 2:4, 0:QB])
                nc.sync.dma_start(wq_t[1][:], wq_r[:, 4:8])
                nc.sync.dma_start(xq0_t[2][:], xqT_r[:, 4:6, 0:QB])
                nc.sync.dma_start(xq0_t[3][:], xqT_r[:, 6:8, 0:QB])
                nc.sync.dma_start(bv_c[:], bv.rearrange("(j p) -> p j", p=P))
                xqr_sb = xqp.tile([P, DC, (NJ - 1) * QB], BF16, name="x_qr")
                nc.sync.dma_start(xqr_sb[:], xqT_r[:, :, QB:seq])
                # two big K drains (one per head-pair) so the psum banks
                # release fast and the psq pool can open early
                nc.scalar.activation(
                    kt_p[0][:], psk[0].rearrange("p q s -> p (q s)"),
                    Act.Identity, bias=bk_sb[:, 0:1],
                )
                nc.vector.tensor_scalar_add(
                    kt_p[1][:], psk[1].rearrange("p q s -> p (q s)"),
                    bk_sb[:, 1:2],
                )

            # ---- long-lived pipeline pools
            pss_pool = tc.tile_pool(name="ps_s", bufs=2, space="PSUM")
            pssp = pss_pool.__enter__()
            ep_pool = tc.tile_pool(name="epool", bufs=6)
            ep = ep_pool.__enter__()
            psq_pool = tc.tile_pool(name="ps_q", bufs=2, space="PSUM",
                                    side="right")
            psqp = psq_pool.__enter__()

            def emit_q(qc, first):
                psq = [psqp.tile([P, QB], F32, tag="q", name=f"psq_{qc}_{j}")
                       for j in range(NP)]
                jorder = [(j, dc) for j in range(NP) for dc in range(DC)]
                if qc != 0:
                    jorder = [(j, dc) for dc in range(DC) for j in range(NP)]
                for j, dc in jorder:
                    rhs = (xq0_t[dc // 2][:, dc % 2] if qc == 0 else
                           xqr_sb[:, dc, (qc - 1) * QB:qc * QB])
                    nc.tensor.matmul(
                        psq[j][:],
                        wq_t[dc // 4][:, dc % 4, j * P:(j + 1) * P],
                        rhs,
                        start=(dc == 0), stop=(dc == DC - 1),
                    )
                for j in range(NP):
                    if first and j == 0:
                        nc.scalar.activation(
                            qt_t[j][qc][:], psq[j][:],
                            Act.Identity, bias=bq_sb[:, j:j + 1],
                        )
                    else:
                        nc.vector.tensor_scalar_add(
                            qt_t[j][qc][:], psq[j][:], bq_sb[:, j:j + 1],
                        )

            def emit_scores(j, h, filler=None):
                hp, hj = h % 2, h // 2
                prow = slice(hp * DK, (hp + 1) * DK)
                e2 = ep.tile([P, K2, 2 * QB], BF16, tag="E",
                             name=f"e2_{j}_{h}")
                for k2 in range(K2):
                    pss = pssp.tile([P, 2 * QB], F32, tag="s",
                                    name=f"pss_{j}_{h}_{k2}")
                    for half in range(2):
                        kt = 2 * k2 + half
                        nc.tensor.matmul(
                            pss[:, half * QB:(half + 1) * QB],
                            kt_p[hj][prow, kt * P:(kt + 1) * P],
                            qt_t[hj][j][prow, :],
                            start=True, stop=True,
                        )
                    nc.scalar.activation(
                        e2[:, k2], pss[:], Act.Exp, scale=inv_sqrt_s,
                    )
                    if filler is not None:
                        filler()
                return e2

            # ---- emission: Q early, scores ASAP, V woven into S1..S3
            # at scores-group granularity (PE slack hides V's 15us under
            # the ACT-paced exp stream without blocking it)
            e2_of = {}
            blocks = [(j, h) for j in range(NJ) for h in range(HPC)]
            pending = []

            emit_q(0, first=True)
            j0, h0 = blocks[0]
            e2_of[(j0, h0)] = emit_scores(j0, h0)
            pending.append((j0, h0))
            with tc.high_priority(offset=-1_000_000):
                emit_q(1, first=False)

            # V pools + DMAs (xv queues right behind xq)
            xv_pool = tc.tile_pool(name="xc_v", bufs=1, side="right")
            wv_pool = tc.tile_pool(name="wp_v", bufs=1, side="right")
            psv_pool = tc.tile_pool(name="ps_v", bufs=2, space="PSUM",
                                    side="right")
            xvp = xv_pool.__enter__()
            wvp = wv_pool.__enter__()
            psvp = psv_pool.__enter__()
            wv_r = wv.rearrange("(dc p) c -> p dc c", p=P)
            xv_dc, wv_dc = [], []
            for dc in range(DC):
                xt = xvp.tile([P, seq], BF16, tag=f"x{dc}", name=f"x_v{dc}")
                nc.sync.dma_start(xt[:], xvT_r[:, dc])
                xv_dc.append(xt)
                wt = wvp.tile([P, CPC], BF16, tag=f"w{dc}", name=f"w_v{dc}")
                nc.sync.dma_start(wt[:], wv_r[:, dc])
                wv_dc.append(wt)

            # V piece list: ("mm", vp, dc) x32 + ("fin", vp) x4
            v_pieces = []
            for vp in range(4):
                v_pieces.extend(("mm", vp, dc) for dc in range(DC))
                v_pieces.append(("fin", vp))
            psv_tiles = {}
            v_idx = [0]
            n_groups = 3 * K2  # filler spread over S1..S3

            def emit_v_piece(kind, vp, dc=0):
                if kind == "mm":
                    if dc == 0:
                        psv_tiles[vp] = [
                            psvp.tile([P, 2, CPC], F32, tag="v",
                                      name=f"psv_{vp}_{i}")
                            for i in range(2)]
                    psv = psv_tiles[vp]
                    for kk in range(4):
                        kt = 4 * vp + kk
                        nc.tensor.matmul(
                            psv[kk // 2][:, kk % 2],
                            xv_dc[dc][:, kt * P:(kt + 1) * P],
                            wv_dc[dc][:, :],
                            start=(dc == 0 and kk % 2 == 0),
                            stop=(dc == DC - 1 and kk % 2 == 1),
                        )
                else:
                    # bv is folded into the at_j drain via the softmax
                    # identity sum(E (V+bv))/den = sum(E V)/den + bv
                    psv = psv_tiles[vp]
                    for kk in range(4):
                        kt = 4 * vp + kk
                        nc.vector.tensor_copy(
                            v_sb[:, kt, :, 0:DK],
                            psv[kk // 2][:, kk % 2].rearrange(
                                "p (h d) -> p h d", h=HPC),
                        )

            group_no = [0]

            def v_filler():
                group_no[0] += 1
                target = (len(v_pieces) * group_no[0]) // n_groups
                with tc.high_priority(offset=-400):
                    while v_idx[0] < min(target, len(v_pieces)):
                        emit_v_piece(*v_pieces[v_idx[0]])
                        v_idx[0] += 1

            for b in (1, 2, 3):
                j, h = blocks[b]
                e2_of[(j, h)] = emit_scores(j, h, filler=v_filler)
                pending.append((j, h))
            with tc.high_priority(offset=-400):
                while v_idx[0] < len(v_pieces):  # finish any leftovers
                    emit_v_piece(*v_pieces[v_idx[0]])
                    v_idx[0] += 1
            psv_pool.__exit__(None, None, None)
            wv_pool.__exit__(None, None, None)
            xv_pool.__exit__(None, None, None)

            def emit_s(b):
                j, h = blocks[b]
                e2_of[(j, h)] = emit_scores(j, h)
                pending.append((j, h))

            # ---- attnV + normalize + transpose pools (2 spare banks)
            psa_pool = tc.tile_pool(name="ps_a", bufs=1, space="PSUM")
            ptr_pool = tc.tile_pool(name="ps_t", bufs=1, space="PSUM")
            asb_pool = tc.tile_pool(name="asb", bufs=8)
            rc_pool = tc.tile_pool(name="rc", bufs=4)
            psap = psa_pool.__enter__()
            ptrp = ptr_pool.__enter__()
            asbp = asb_pool.__enter__()
            rcp = rc_pool.__enter__()
            psop = op = wo_sb = None

            def emit_attnv(j, h, pair, half):
                e2 = e2_of.pop((j, h))
                psa = psap.tile([P, QT, DK + 4], F32, tag="a",
                                name=f"psa_{j}_{h}")
                for kt in range(KT):
                    for g in range(QT):
                        q0 = (kt % 2) * QB + g * P
                        nc.tensor.matmul(
                            psa[:, g, 0:DK + 1],
                            e2[:, kt // 2, q0:q0 + P],
                            v_sb[:, kt, h, :],
                            start=(g == 0 and kt == 0),
                            stop=(g == QT - 1 and kt == KT - 1),
                        )
                for g in range(QT):
                    rc = rcp.tile([P, 1], F32, tag="rc")
                    nc.vector.reciprocal(rc[:], psa[:, g, DK:DK + 1])
                    nc.vector.tensor_scalar_mul(
                        pair[g][:, half], psa[:, g, 0:DK], rc[:])

            def emit_transpose_g(j, hj, pair, g, on_act=False):
                ptr = ptrp.tile([P, P], F32, tag="t",
                                name=f"ptr_{j}_{hj}_{g}")
                nc.tensor.matmul(
                    ptr[:], pair[g].rearrange("p h d -> p (h d)"),
                    ident_sb[:], is_transpose=True,
                    start=True, stop=True,
                )
                if on_act:
                    nc.scalar.activation(
                        at_j[j][:, hj, g * P:(g + 1) * P], ptr[:],
                        Act.Identity, bias=bv_c[:, hj:hj + 1])
                else:
                    nc.vector.tensor_scalar_add(
                        at_j[j][:, hj, g * P:(g + 1) * P], ptr[:],
                        bv_c[:, hj:hj + 1])

            def emit_oproj_ql(j, ql, on_act=False):
                qt0 = j * QT + ql
                o_sb = op.tile([P, D], F32, tag="o_sb")
                for nh in range(D // QB):
                    pso = psop.tile([P, QB], F32, tag="o",
                                    name=f"pso_{qt0}_{nh}")
                    for dj in range(NP):
                        nc.tensor.matmul(
                            pso[:],
                            at_j[j][:, dj, ql * P:(ql + 1) * P],
                            wo_sb[:, dj, nh * QB:(nh + 1) * QB],
                            start=(dj == 0), stop=(dj == NP - 1),
                        )
                    if on_act and nh == 0:
                        nc.scalar.activation(
                            o_sb[:, nh * QB:(nh + 1) * QB], pso[:],
                            Act.Identity)
                    else:
                        nc.vector.tensor_copy(
                            o_sb[:, nh * QB:(nh + 1) * QB], pso[:])
                    nc.sync.dma_start(
                        out[qt0 * P:(qt0 + 1) * P,
                            nh * QB:(nh + 1) * QB],
                        o_sb[:, nh * QB:(nh + 1) * QB],
                    )

            asb_h0 = {}
            deferred_o = []

            def pop_one():
                jj, hh = pending.pop(0)
                hj = hh // 2
                if hh % 2 == 0:
                    pair = [asbp.tile([P, 2, DK], F32, tag="asb",
                                      name=f"asb_{jj}_{hj}_{g}")
                            for g in range(QT)]
                    asb_h0[hj] = pair
                    emit_attnv(jj, hh, pair, 0)
                    return
                pair = asb_h0.pop(hj)
                emit_attnv(jj, hh, pair, 1)
                if hh == HPC - 1:
                    # interleave transpose -> at drain -> O proj per ql so
                    # the tail pipeline is as short as possible
                    final = jj == NJ - 1
                    for g in range(QT):
                        emit_transpose_g(jj, hj, pair, g)
                        if psop is None:
                            deferred_o.append(jj)  # once per g; dedup below
                        else:
                            emit_oproj_ql(jj, g, on_act=final)
                else:
                    for g in range(QT):
                        emit_transpose_g(jj, hj, pair, g)

            pop_one()  # A(S0)
            pop_one()  # A(S1)
            for b in range(4, len(blocks)):
                if b == 8:
                    with tc.high_priority(offset=-1_000_000):
                        emit_q(2, first=False)
                if b == 9:
                    with tc.high_priority(offset=-1_000_000):
                        emit_q(3, first=False)
                    psq_pool.__exit__(None, None, None)
                    wq_pool.__exit__(None, None, None)
                    xq_pool.__exit__(None, None, None)
                    # O-projection pools (they need psq's 2 banks)
                    pso_pool = tc.tile_pool(name="ps_o", bufs=2,
                                            space="PSUM")
                    op_pool = tc.tile_pool(name="opool", bufs=2)
                    wo_pool = tc.tile_pool(name="wop", bufs=1)
                    psop = pso_pool.__enter__()
                    op = op_pool.__enter__()
                    wop = wo_pool.__enter__()
                    wo_sb = wop.tile([P, NP, D], BF16)
                    nc.sync.dma_start(
                        wo_sb[:], wo.rearrange("(dj p) n -> p dj n", p=P))
                    with tc.high_priority(offset=-400):
                        for jd in sorted(set(deferred_o)):
                            for g in range(QT):
                                emit_oproj_ql(jd, g)
                    deferred_o.clear()
                emit_s(b)
                while len(pending) > LOOKAHEAD:
                    pop_one()
            while pending:
                pop_one()

            for pool in (wo_pool, op_pool, pso_pool, rc_pool, asb_pool,
                         ptr_pool, psa_pool, ep_pool, pss_pool):
                pool.__exit__(None, None, None)

    nc.compile()
    return nc


_PROGRAM_CACHE = {}


def _get_program(seq=S):
    if seq not in _PROGRAM_CACHE:
        _PROGRAM_CACHE[seq] = build_program(seq)
    return _PROGRAM_CACHE[seq]


def make_in_maps(queries, keys, values, Wq, bq, Wk, bk, Wv, bv, Wo, bo):
    """Per-core input dicts implementing the sharding."""
    import ml_dtypes
    f32 = np.float32
    bf16 = ml_dtypes.bfloat16
    xT = {}
    for b in range(B):
        xT[b] = (
            np.ascontiguousarray(
                np.asarray(queries[b], dtype=f32).T).astype(bf16),
            np.ascontiguousarray(
                np.asarray(keys[b], dtype=f32).T).astype(bf16),
            np.ascontiguousarray(
                np.asarray(values[b], dtype=f32).T).astype(bf16),
        )
    Wq, Wk, Wv, Wo = (np.asarray(a, dtype=f32) for a in (Wq, Wk, Wv, Wo))
    bq, bk, bv = (np.asarray(a, dtype=f32) for a in (bq, bk, bv))
    in_maps = []
    for c in range(NCORES):
        b, g = divmod(c, GROUPS)
        cs = slice(g * CPC, (g + 1) * CPC)
        qT, kT, vT = xT[b]
        in_maps.append({
            "xqT": qT, "xkT": kT, "xvT": vT,
            "wq": np.ascontiguousarray(Wq[:, cs]).astype(bf16),
            "wk": np.ascontiguousarray(Wk[:, cs]).astype(bf16),
            "wv": np.ascontiguousarray(Wv[:, cs]).astype(bf16),
            "wo": np.ascontiguousarray(Wo[cs, :]).astype(bf16),
            "bq": np.ascontiguousarray(bq[cs]),
            "bk": np.ascontiguousarray(bk[cs]),
            "bv": np.ascontiguousarray(bv[cs]),
            "ident": np.eye(P, dtype=f32),
            "vones": np.ones((P, 16 * HPC), dtype=bf16),
        })
    return in_maps


def combine_outputs(results, bo):
    """Host all-reduce of the Wo row-shard partials + bias."""
    bo = np.asarray(bo, dtype=np.float32)
    outs = []
    for b in range(B):
        acc = results[b * GROUPS]["out"].astype(np.float32)
        for g in range(1, GROUPS):
            acc = acc + results[b * GROUPS + g]["out"].astype(np.float32)
        outs.append(acc + bo)
    return np.stack(outs)


def kernel(queries, keys, values, Wq, bq, Wk, bk, Wv, bv, Wo, bo):
    nc = _get_program()
    in_maps = make_in_maps(queries, keys, values, Wq, bq, Wk, bk, Wv, bv,
                           Wo, bo)
    res = run_bass_kernel_spmd(nc, in_maps, list(range(NCORES)))
    return combine_outputs(res.results, bo)


# revision 9
# speedup vs baseline: 1.3830x; 1.0197x over previous
"""MultiHeadAttention forward on 8 Trainium2 NeuronCores (bf16, v3).

Sharding (Megatron-style tensor parallel x data parallel):
  core c (0..7): batch b = c // 4, head group g = c % 4 (4 of 16 heads).
  Wq/Wk/Wv column-sharded ([1024, 256] per core), Wo row-sharded
  ([256, 1024] per core). Each core computes a partial output
  [S, D] = attn(heads g) @ Wo_rows; the host sums the 4 partials per
  batch and adds bo.

The kernel is ACT(exp)-bound: 16.8M exps at 128/cycle/1.2GHz = 133us.
Everything else is scheduled to (a) start the exp stream as early as
the xk+xq DMA allows (~28us) and (b) never starve it:
  - bf16 everywhere (DMA halved, matmuls full rate)
  - warm-up matmuls on zeros ramp the PE p-state before xk lands
  - DMA order xk, xq, xv; K is chunk-streamed, Q is chunk-resident and
    computed q-block-0-first so scores start right after xq lands
  - V projection in 4 passes of 4 k-tiles on 2 PSUM banks, emitted
    consecutively after the first 4 scores blocks (its PE time hides
    under the ACT-paced pipeline)
  - attnV flipped to [q-part, dk+1] (lhsT = E tile): half the PE rows,
    normalize is reciprocal + per-partition tensor_scalar on DVE, and a
    PE transpose rebuilds [c, q] for the O projection
PSUM banks (left|right): warm1|0 -> K8|0 -> pss4|psq2 -> pss4|psq2+
psv2 -> pss4+psa1+ptr1|psq2 -> pss4+psa1+ptr1+pso2|0.
"""

import math

import numpy as np

import concourse.bass as bass
import concourse.mybir as mybir
import concourse.tile as tile
from concourse import bacc
from concourse.bass_utils import run_bass_kernel_spmd

P = 128
B, S, D, H = 2, 2048, 1024, 16
NCORES = 8
GROUPS = NCORES // B          # 4 head-groups
HPC = H // GROUPS             # 4 heads per core
DK = D // H                   # 64
CPC = HPC * DK                # 256 cols per core
NP = CPC // P                 # 2 head pairs per core
DC = D // P                   # 8 contraction chunks over D
QB = 512                      # q block (matmul moving free dim)
QT = QB // P                  # 4 q-subtiles per block

WARMUP = 10                   # p-state ramp matmuls on zeros
LOOKAHEAD = 3                 # scored blocks in flight before attnV

F32 = mybir.dt.float32
BF16 = mybir.dt.bfloat16
Act = mybir.ActivationFunctionType


def build_program(seq=S):
    KT = seq // P             # 16 k tiles
    NJ = seq // QB            # 4 q blocks
    K2 = KT // 2
    inv_sqrt_s = 1.0 / math.sqrt(S)

    nc = bacc.Bacc("TRN2", target_bir_lowering=False, debug=False,
                   num_devices=NCORES)
    xqT = nc.declare_dram_parameter("xqT", [D, seq], BF16, isOutput=False)
    xkT = nc.declare_dram_parameter("xkT", [D, seq], BF16, isOutput=False)
    xvT = nc.declare_dram_parameter("xvT", [D, seq], BF16, isOutput=False)
    wq = nc.declare_dram_parameter("wq", [D, CPC], BF16, isOutput=False)
    wk = nc.declare_dram_parameter("wk", [D, CPC], BF16, isOutput=False)
    wv = nc.declare_dram_parameter("wv", [D, CPC], BF16, isOutput=False)
    wo = nc.declare_dram_parameter("wo", [CPC, D], BF16, isOutput=False)
    bq = nc.declare_dram_parameter("bq", [CPC], F32, isOutput=False)
    bk = nc.declare_dram_parameter("bk", [CPC], F32, isOutput=False)
    bv = nc.declare_dram_parameter("bv", [CPC], F32, isOutput=False)
    ident = nc.declare_dram_parameter("ident", [P, P], F32, isOutput=False)
    vones = nc.declare_dram_parameter("vones", [P, KT * HPC], BF16,
                                      isOutput=False)
    out = nc.declare_dram_parameter("out", [seq, D], F32, isOutput=True)

    xqT_r = xqT.rearrange("(dc p) s -> p dc s", p=P)
    xkT_r = xkT.rearrange("(dc p) s -> p dc s", p=P)
    xvT_r = xvT.rearrange("(dc p) s -> p dc s", p=P)

    with tile.TileContext(nc) as tc:
        with tc.tile_pool(name="consts", bufs=1) as consts:
            bq_sb = consts.tile([P, NP], F32)
            bk_sb = consts.tile([P, NP], F32)
            bv_c = consts.tile([P, NP], F32)
            ident_sb = consts.tile([P, P], F32)
            scratch = consts.tile([P, 1], F32)
            zw = consts.tile([P, P], BF16)
            zx = consts.tile([P, QB], BF16)
            nc.vector.memset(zw[:], 0.0)
            nc.vector.memset(zx[:], 0.0)

            # Persistent activations
            qt_t = [[consts.tile([P, QB], BF16, name=f"qt_{j}_{qc}")
                     for qc in range(NJ)] for j in range(NP)]
            kt_p = [consts.tile([P, seq], BF16, name=f"kt_p{j}")
                    for j in range(NP)]
            v_sb = consts.tile([P, KT, HPC, DK + 1], BF16)
            at_j = [consts.tile([P, NP, QB], BF16, name=f"at_j{j}")
                    for j in range(NJ)]
            # softmax-denominator ones column of V
            nc.vector.memset(v_sb[:, :, :, DK:DK + 1], 1.0)

            # p-state warm-up: PE runs zeros while the first xk chunk
            # streams, so the real matmuls start at full clock
            with tc.tile_pool(name="warm", bufs=1, space="PSUM") as warmp:
                wps = warmp.tile([P, QB], F32, tag="wps")
                for _ in range(WARMUP):
                    nc.tensor.matmul(wps[:], zw[:], zx[:],
                                     start=True, stop=True)

            # ---- K projection (xk-streamed, 8 psum banks)
            with (
                tc.tile_pool(name="xc_k", bufs=6) as xkp,
                tc.tile_pool(name="wp_k", bufs=1) as wkp,
                tc.tile_pool(name="ps_k", bufs=1, space="PSUM") as pskp,
            ):
                wk_r = wk.rearrange("(dc p) c -> p dc c", p=P)
                wk_dc = [wkp.tile([P, CPC], BF16, tag=f"w{dc}",
                                  name=f"w_k{dc}") for dc in range(DC)]
                psk = [pskp.tile([P, NJ, QB], F32, tag=f"qk{j}",
                                 name=f"psk_{j}")
                       for j in range(NP)]
                xk_t = []
                for dc in range(DC):
                    xt = xkp.tile([P, seq], BF16, tag="xc")
                    nc.sync.dma_start(xt[:], xkT_r[:, dc])
                    nc.sync.dma_start(wk_dc[dc][:], wk_r[:, dc])
                    xk_t.append(xt)
                    if dc == 0:
                        # small consts ride the queue behind the first chunk
                        nc.sync.dma_start(
                            bk_sb[:], bk.rearrange("(j p) -> p j", p=P))
                        nc.sync.dma_start(
                            bq_sb[:], bq.rearrange("(j p) -> p j", p=P))
                        nc.sync.dma_start(ident_sb[:], ident[:])
                        # dummy exp: preloads the ACT function table
                        nc.scalar.activation(scratch[:], bk_sb[:, 0:1],
                                             Act.Exp)
                    for j in range(NP):
                        for qc in range(NJ):
                            nc.tensor.matmul(
                                psk[j][:, qc],
                                wk_dc[dc][:, j * P:(j + 1) * P],
                                xt[:, qc * QB:(qc + 1) * QB],
                                start=(dc == 0), stop=(dc == DC - 1),
                            )
                # xq DMAs queue right behind xk; q-block-0's columns are
                # split out and land first so scores start ~10us earlier
                xq_pool = tc.tile_pool(name="xc_q", bufs=1, side="right")
                wq_pool = tc.tile_pool(name="wp_q", bufs=1, side="right")
                xqp = xq_pool.__enter__()
                wqp = wq_pool.__enter__()
                wq_r = wq.rearrange("(dc p) c -> p dc c", p=P)
                # separate tiles per DMA piece: Tile tracks deps per tile,
                # so q0's early dc-chunks become usable as they land
                wq_t = [wqp.tile([P, 4, CPC], BF16, name=f"w_q{i}")
                        for i in range(2)]
                xq0_t = [xqp.tile([P, 2, QB], BF16, name=f"x_q0_{i}")
                         for i in range(4)]
                nc.sync.dma_start(wq_t[0][:], wq_r[:, 0:4])
                nc.sync.dma_start(xq0_t[0][:], xqT_r[:, 0:2, 0:QB])
                nc.sync.dma_start(xq0_t[1][:], xqT_r[:, 2:4, 0:QB])
                nc.sync.dma_start(wq_t[1][:], wq_r[:, 4:8])
                nc.sync.dma_start(xq0_t[2][:], xqT_r[:, 4:6, 0:QB])
                nc.sync.dma_start(xq0_t[3][:], xqT_r[:, 6:8, 0:QB])
                nc.sync.dma_start(bv_c[:], bv.rearrange("(j p) -> p j", p=P))
                xqr_sb = xqp.tile([P, DC, (NJ - 1) * QB], BF16, name="x_qr")
                nc.sync.dma_start(xqr_sb[:], xqT_r[:, :, QB:seq])
                # two big K drains (one per head-pair) so the psum banks
                # release fast and the psq pool can open early
                nc.scalar.activation(
                    kt_p[0][:], psk[0].rearrange("p q s -> p (q s)"),
                    Act.Identity, bias=bk_sb[:, 0:1],
                )
                nc.vector.tensor_scalar_add(
                    kt_p[1][:], psk[1].rearrange("p q s -> p (q s)"),
                    bk_sb[:, 1:2],
                )

            # ---- long-lived pipeline pools
            pss_pool = tc.tile_pool(name="ps_s", bufs=2, space="PSUM")
            pssp = pss_pool.__enter__()
            ep_pool = tc.tile_pool(name="epool", bufs=6)
            ep = ep_pool.__enter__()
            psq_pool = tc.tile_pool(name="ps_q", bufs=2, space="PSUM",
                                    side="right")
            psqp = psq_pool.__enter__()

            def emit_q(qc, first):
                psq = [psqp.tile([P, QB], F32, tag="q", name=f"psq_{qc}_{j}")
                       for j in range(NP)]
                jorder = [(j, dc) for j in range(NP) for dc in range(DC)]
                if qc != 0:
                    jorder = [(j, dc) for dc in range(DC) for j in range(NP)]
                for j, dc in jorder:
                    rhs = (xq0_t[dc // 2][:, dc % 2] if qc == 0 else
                           xqr_sb[:, dc, (qc - 1) * QB:qc * QB])
                    nc.tensor.matmul(
                        psq[j][:],
                        wq_t[dc // 4][:, dc % 4, j * P:(j + 1) * P],
                        rhs,
                        start=(dc == 0), stop=(dc == DC - 1),
                    )
                for j in range(NP):
                    if first and j == 0:
                        nc.scalar.activation(
                            qt_t[j][qc][:], psq[j][:],
                            Act.Identity, bias=bq_sb[:, j:j + 1],
                        )
                    else:
                        nc.vector.tensor_scalar_add(
                            qt_t[j][qc][:], psq[j][:], bq_sb[:, j:j + 1],
                        )

            def emit_scores(j, h, filler=None):
                hp, hj = h % 2, h // 2
                prow = slice(hp * DK, (hp + 1) * DK)
                e2 = ep.tile([P, K2, 2 * QB], BF16, tag="E",
                             name=f"e2_{j}_{h}")
                for k2 in range(K2):
                    pss = pssp.tile([P, 2 * QB], F32, tag="s",
                                    name=f"pss_{j}_{h}_{k2}")
                    for half in range(2):
                        kt = 2 * k2 + half
                        nc.tensor.matmul(
                            pss[:, half * QB:(half + 1) * QB],
                            kt_p[hj][prow, kt * P:(kt + 1) * P],
                            qt_t[hj][j][prow, :],
                            start=True, stop=True,
                        )
                    nc.scalar.activation(
                        e2[:, k2], pss[:], Act.Exp, scale=inv_sqrt_s,
                    )
                    if filler is not None:
                        filler()
                return e2

            # ---- emission: Q early, scores ASAP, V woven into S1..S3
            # at scores-group granularity (PE slack hides V's 15us under
            # the ACT-paced exp stream without blocking it)
            e2_of = {}
            blocks = [(j, h) for j in range(NJ) for h in range(HPC)]
            pending = []

            emit_q(0, first=True)
            j0, h0 = blocks[0]
            e2_of[(j0, h0)] = emit_scores(j0, h0)
            pending.append((j0, h0))
            with tc.high_priority(offset=-1_000_000):
                emit_q(1, first=False)

            # V pools + DMAs (xv queues right behind xq)
            xv_pool = tc.tile_pool(name="xc_v", bufs=1, side="right")
            wv_pool = tc.tile_pool(name="wp_v", bufs=1, side="right")
            psv_pool = tc.tile_pool(name="ps_v", bufs=2, space="PSUM",
                                    side="right")
            xvp = xv_pool.__enter__()
            wvp = wv_pool.__enter__()
            psvp = psv_pool.__enter__()
            wv_r = wv.rearrange("(dc p) c -> p dc c", p=P)
            xv_dc, wv_dc = [], []
            for dc in range(DC):
                xt = xvp.tile([P, seq], BF16, tag=f"x{dc}", name=f"x_v{dc}")
                nc.sync.dma_start(xt[:], xvT_r[:, dc])
                xv_dc.append(xt)
                wt = wvp.tile([P, CPC], BF16, tag=f"w{dc}", name=f"w_v{dc}")
                nc.sync.dma_start(wt[:], wv_r[:, dc])
                wv_dc.append(wt)

            # V piece list: ("mm", vp, dc) x32 + ("fin", vp) x4
            v_pieces = []
            for vp in range(4):
                v_pieces.extend(("mm", vp, dc) for dc in range(DC))
                v_pieces.append(("fin", vp))
            psv_tiles = {}
            v_idx = [0]
            n_groups = 3 * K2  # filler spread over S1..S3

            def emit_v_piece(kind, vp, dc=0):
                if kind == "mm":
                    if dc == 0:
                        psv_tiles[vp] = [
                            psvp.tile([P, 2, CPC], F32, tag="v",
                                      name=f"psv_{vp}_{i}")
                            for i in range(2)]
                    psv = psv_tiles[vp]
                    for kk in range(4):
                        kt = 4 * vp + kk
                        nc.tensor.matmul(
                            psv[kk // 2][:, kk % 2],
                            xv_dc[dc][:, kt * P:(kt + 1) * P],
                            wv_dc[dc][:, :],
                            start=(dc == 0 and kk % 2 == 0),
                            stop=(dc == DC - 1 and kk % 2 == 1),
                        )
                else:
                    # bv is folded into the at_j drain via the softmax
                    # identity sum(E (V+bv))/den = sum(E V)/den + bv
                    psv = psv_tiles[vp]
                    for kk in range(4):
                        kt = 4 * vp + kk
                        nc.vector.tensor_copy(
                            v_sb[:, kt, :, 0:DK],
                            psv[kk // 2][:, kk % 2].rearrange(
                                "p (h d) -> p h d", h=HPC),
                        )

            group_no = [0]

            def v_filler():
                group_no[0] += 1
                target = (len(v_pieces) * group_no[0]) // n_groups
                with tc.high_priority(offset=-400):
                    while v_idx[0] < min(target, len(v_pieces)):
                        emit_v_piece(*v_pieces[v_idx[0]])
                        v_idx[0] += 1

            for b in (1, 2, 3):
                j, h = blocks[b]
                e2_of[(j, h)] = emit_scores(j, h, filler=v_filler)
                pending.append((j, h))
            with tc.high_priority(offset=-400):
                while v_idx[0] < len(v_pieces):  # finish any leftovers
                    emit_v_piece(*v_pieces[v_idx[0]])
                    v_idx[0] += 1
            psv_pool.__exit__(None, None, None)
            wv_pool.__exit__(None, None, None)
            xv_pool.__exit__(None, None, None)

            def emit_s(b):
                j, h = blocks[b]
                e2_of[(j, h)] = emit_scores(j, h)
                pending.append((j, h))

            # ---- attnV + normalize + transpose pools (2 spare banks)
            psa_pool = tc.tile_pool(name="ps_a", bufs=1, space="PSUM")
            ptr_pool = tc.tile_pool(name="ps_t", bufs=1, space="PSUM")
            asb_pool = tc.tile_pool(name="asb", bufs=8)
            rc_pool = tc.tile_pool(name="rc", bufs=4)
            psap = psa_pool.__enter__()
            ptrp = ptr_pool.__enter__()
            asbp = asb_pool.__enter__()
            rcp = rc_pool.__enter__()
            psop = op = wo_sb = None

            def emit_attnv(j, h, pair, half):
                e2 = e2_of.pop((j, h))
                psa = psap.tile([P, QT, DK + 4], F32, tag="a",
                                name=f"psa_{j}_{h}")
                for kt in range(KT):
                    for g in range(QT):
                        q0 = (kt % 2) * QB + g * P
                        nc.tensor.matmul(
                            psa[:, g, 0:DK + 1],
                            e2[:, kt // 2, q0:q0 + P],
                            v_sb[:, kt, h, :],
                            start=(g == 0 and kt == 0),
                            stop=(g == QT - 1 and kt == KT - 1),
                        )
                for g in range(QT):
                    rc = rcp.tile([P, 1], F32, tag="rc")
                    nc.vector.reciprocal(rc[:], psa[:, g, DK:DK + 1])
                    nc.vector.tensor_scalar_mul(
                        pair[g][:, half], psa[:, g, 0:DK], rc[:])

            def emit_transpose_g(j, hj, pair, g, on_act=False):
                ptr = ptrp.tile([P, P], F32, tag="t",
                                name=f"ptr_{j}_{hj}_{g}")
                nc.tensor.matmul(
                    ptr[:], pair[g].rearrange("p h d -> p (h d)"),
                    ident_sb[:], is_transpose=True,
                    start=True, stop=True,
                )
                if on_act:
                    nc.scalar.activation(
                        at_j[j][:, hj, g * P:(g + 1) * P], ptr[:],
                        Act.Identity, bias=bv_c[:, hj:hj + 1])
                else:
                    nc.vector.tensor_scalar_add(
                        at_j[j][:, hj, g * P:(g + 1) * P], ptr[:],
                        bv_c[:, hj:hj + 1])

            def emit_oproj_ql(j, ql, on_act=False):
                qt0 = j * QT + ql
                o_sb = op.tile([P, D], F32, tag="o_sb")
                for nh in range(D // QB):
                    pso = psop.tile([P, QB], F32, tag="o",
                                    name=f"pso_{qt0}_{nh}")
                    for dj in range(NP):
                        nc.tensor.matmul(
                            pso[:],
                            at_j[j][:, dj, ql * P:(ql + 1) * P],
                            wo_sb[:, dj, nh * QB:(nh + 1) * QB],
                            start=(dj == 0), stop=(dj == NP - 1),
                        )
                    if on_act and nh == 0:
                        nc.scalar.activation(
                            o_sb[:, nh * QB:(nh + 1) * QB], pso[:],
                            Act.Identity)
                    else:
                        nc.vector.tensor_copy(
                            o_sb[:, nh * QB:(nh + 1) * QB], pso[:])
                    nc.sync.dma_start(
                        out[qt0 * P:(qt0 + 1) * P,
                            nh * QB:(nh + 1) * QB],
                        o_sb[:, nh * QB:(nh + 1) * QB],
                    )

            asb_h0 = {}
            deferred_o = []

            def pop_one():
                jj, hh = pending.pop(0)
                hj = hh // 2
                if hh % 2 == 0:
                    pair = [asbp.tile([P, 2, DK], F32, tag="asb",
                                      name=f"asb_{jj}_{hj}_{g}")
                            for g in range(QT)]
                    asb_h0[hj] = pair
                    emit_attnv(jj, hh, pair, 0)
                    return
                pair = asb_h0.pop(hj)
                emit_attnv(jj, hh, pair, 1)
                if hh == HPC - 1:
                    # interleave transpose -> at drain -> O proj per ql so
                    # the tail pipeline is as short as possible
                    final = jj == NJ - 1
                    for g in range(QT):
                        emit_transpose_g(jj, hj, pair, g)
                        if psop is None:
                            deferred_o.append(jj)  # once per g; dedup below
                        else:
                            emit_oproj_ql(jj, g, on_act=final)
                else:
                    for g in range(QT):
                        emit_transpose_g(jj, hj, pair, g)

            pop_one()  # A(S0)
            pop_one()  # A(S1)
            for b in range(4, len(blocks)):
                if b == 8:
                    with tc.high_priority(offset=-1_000_000):
                        emit_q(2, first=False)
                if b == 9:
                    with tc.high_priority(offset=-1_000_000):
                        emit_q(3, first=False)
                    psq_pool.__exit__(None, None, None)
                    wq_pool.__exit__(None, None, None)
                    xq_pool.__exit__(None, None, None)
                    # O-projection pools (they need psq's 2 banks)
                    pso_pool = tc.tile_pool(name="ps_o", bufs=2,
                                            space="PSUM")
                    op_pool = tc.tile_pool(name="opool", bufs=2)
                    wo_pool = tc.tile_pool(name="wop", bufs=1)
                    psop = pso_pool.__enter__()
                    op = op_pool.__enter__()
                    wop = wo_pool.__enter__()
                    wo_sb = wop.tile([P, NP, D], BF16)
                    nc.sync.dma_start(
                        wo_sb[:], wo.rearrange("(dj p) n -> p dj n", p=P))
                    with tc.high_priority(offset=-400):
                        for jd in sorted(set(deferred_o)):
                            for g in range(QT):
                                emit_oproj_ql(jd, g)
                    deferred_o.clear()
                emit_s(b)
                while len(pending) > LOOKAHEAD:
                    pop_one()
            while pending:
                pop_one()

            for pool in (wo_pool, op_pool, pso_pool, rc_pool, asb_pool,
                         ptr_pool, psa_pool, ep_pool, pss_pool):
                pool.__exit__(None, None, None)

    nc.compile()
    return nc


_PROGRAM_CACHE = {}


def _get_program(seq=S):
    if seq not in _PROGRAM_CACHE:
        _PROGRAM_CACHE[seq] = build_program(seq)
    return _PROGRAM_CACHE[seq]


def make_in_maps(queries, keys, values, Wq, bq, Wk, bk, Wv, bv, Wo, bo):
    """Per-core input dicts implementing the sharding."""
    import ml_dtypes
    f32 = np.float32
    bf16 = ml_dtypes.bfloat16
    xT = {}
    for b in range(B):
        xT[b] = (
            np.ascontiguousarray(
                np.asarray(queries[b], dtype=f32).T).astype(bf16),
            np.ascontiguousarray(
                np.asarray(keys[b], dtype=f32).T).astype(bf16),
            np.ascontiguousarray(
                np.asarray(values[b], dtype=f32).T).astype(bf16),
        )
    Wq, Wk, Wv, Wo = (np.asarray(a, dtype=f32) for a in (Wq, Wk, Wv, Wo))
    bq, bk, bv = (np.asarray(a, dtype=f32) for a in (bq, bk, bv))
    in_maps = []
    for c in range(NCORES):
        b, g = divmod(c, GROUPS)
        cs = slice(g * CPC, (g + 1) * CPC)
        qT, kT, vT = xT[b]
        in_maps.append({
            "xqT": qT, "xkT": kT, "xvT": vT,
            "wq": np.ascontiguousarray(Wq[:, cs]).astype(bf16),
            "wk": np.ascontiguousarray(Wk[:, cs]).astype(bf16),
            "wv": np.ascontiguousarray(Wv[:, cs]).astype(bf16),
            "wo": np.ascontiguousarray(Wo[cs, :]).astype(bf16),
            "bq": np.ascontiguousarray(bq[cs]),
            "bk": np.ascontiguousarray(bk[cs]),
            "bv": np.ascontiguousarray(bv[cs]),
            "ident": np.eye(P, dtype=f32),
            "vones": np.ones((P, 16 * HPC), dtype=bf16),
        })
    return in_maps


def combine_outputs(results, bo):
    """Host all-reduce of the Wo row-shard partials + bias."""
    bo = np.asarray(bo, dtype=np.float32)
    outs = []
    for b in range(B):
        acc = results[b * GROUPS]["out"].astype(np.float32)
        for g in range(1, GROUPS):
            acc = acc + results[b * GROUPS + g]["out"].astype(np.float32)
        outs.append(acc + bo)
    return np.stack(outs)


def kernel(queries, keys, values, Wq, bq, Wk, bk, Wv, bv, Wo, bo):
    nc = _get_program()
    in_maps = make_in_maps(queries, keys, values, Wq, bq, Wk, bk, Wv, bv,
                           Wo, bo)
    res = run_bass_kernel_spmd(nc, in_maps, list(range(NCORES)))
    return combine_outputs(res.results, bo)
